# revision 1
# baseline (speedup 1.0000x reference)
"""Trainium2 Bass kernel for nn_MHA_36584531427723.

Sharding: 8 cores = 2 batches x 4 head-groups (4 heads of 64 dims each per
core). Each core computes its batch's Q/K/V projections restricted to its
head-group's 256 output features, attention for its 4 heads, and a partial
output projection (its 256 rows of Wo^T). The host sums the 4 partials per
batch and adds bo.

Device layout choices (all host-prepped, no on-device transposes):
  - QT/KT = Q[b].T, K[b].T   [1024, 2048] f32  (feature on partitions)
  - projections produce Q_^T/K_^T [256, 2048] (bf16) and V [2048, 4, 65] bf16
    with a ones column at index 64 so the PV matmul also yields the softmax
    denominator row.
  - scores are computed transposed, E^T [k, q], so exp/mask/PV all use
    natural slices; mask is shipped pre-transposed as bf16 0/1.
  - softmax: max-subtraction dropped (|E|<~1 so exp is safe; the reference's
    max shift cancels exactly up to its eps term, relative effect ~1e-11);
    eps dropped (eps/S ~ 1e-11).
"""

import numpy as np
import ml_dtypes

import concourse.bacc as bacc
import concourse.bass as bass  # noqa: F401
import concourse.mybir as mybir
import concourse.tile as tile
from concourse.bass_utils import run_bass_kernel_spmd

B, N, D = 2, 2048, 1024
H = 16
HD = 64
HL = 4  # heads per core
DL = HL * HD  # 256 local features
P = 128
KO = D // P  # 8 contraction chunks for projections
NKC = N // P  # 16 k-token chunks
NQC = N // P
NPAN = 4
PANW = N // NPAN  # 512-wide token panels in the projection phase
SCALE = 1.0 / 32.0  # 1/sqrt(DIM_V)

F32 = mybir.dt.float32
F32R = mybir.dt.float32r
BF16 = mybir.dt.bfloat16
AF = mybir.ActivationFunctionType

def build_nc():
    nc = bacc.Bacc(None, target_bir_lowering=False)
    QT = nc.dram_tensor("qt", (D, N), BF16, kind="ExternalInput")
    KT = nc.dram_tensor("kt", (D, N), BF16, kind="ExternalInput")
    MT = nc.dram_tensor("mt", (N, N), BF16, kind="ExternalInput")
    WQT = nc.dram_tensor("wqt", (D, DL), BF16, kind="ExternalInput")
    WKT = nc.dram_tensor("wkt", (D, DL), BF16, kind="ExternalInput")
    WVT = nc.dram_tensor("wvt", (D, DL), BF16, kind="ExternalInput")
    WOT = nc.dram_tensor("wot", (DL, D), BF16, kind="ExternalInput")
    BQ = nc.dram_tensor("bq", (DL,), F32, kind="ExternalInput")
    BK = nc.dram_tensor("bk", (DL,), F32, kind="ExternalInput")
    BV = nc.dram_tensor("bv", (DL,), F32, kind="ExternalInput")
    OUT = nc.dram_tensor("out", (N, D), F32, kind="ExternalOutput")

    qt_r = QT[:].rearrange("(ko p) q -> p ko q", p=P)
    kt_r = KT[:].rearrange("(ko p) q -> p ko q", p=P)
    mt_r = MT[:].rearrange("(kc p) q -> p kc q", p=P)

    with tile.TileContext(nc) as tc:
        with (
            tc.tile_pool(name="persist", bufs=1) as persist,
            tc.tile_pool(name="otpool", bufs=1) as otpool,
        ):
            # --- persistent tiles ---
            mT = persist.tile([P, NKC, N], BF16)  # 64KB/part
            qT = persist.tile([P, 2, N], BF16, tag="qT")  # Q_^T, 8KB
            kT = persist.tile([P, 2, N], BF16, tag="kT")
            v_sb = persist.tile([P, NKC, HL, HD + 1], BF16, tag="v")
            ones_sb = persist.tile([1, HD], F32, tag="ones")
            nc.vector.memset(ones_sb[:], 1.0)
            bq_sb = persist.tile([P, 2], F32, tag="bq")
            bk_sb = persist.tile([P, 2], F32, tag="bk")
            bv_rep = persist.tile([P, HL, HD], F32, tag="bv")
            wo_sb = persist.tile([P, 2, D], BF16, tag="wo")

            nc.sync.dma_start(out=bq_sb[:], in_=BQ[:].rearrange("(c p) -> p c", p=P))
            nc.sync.dma_start(out=bk_sb[:], in_=BK[:].rearrange("(c p) -> p c", p=P))
            nc.sync.dma_start(
                out=bv_rep[:],
                in_=BV[:].rearrange("(h d) -> h d", h=HL)[None].to_broadcast(
                    (P, HL, HD)
                ),
            )
            for cc in range(2):
                nc.sync.dma_start(
                    out=wo_sb[:, cc, :],
                    in_=WOT[:].rearrange("(cc p) n -> p cc n", p=P)[:, cc, :],
                )
            nc.vector.memset(v_sb[:, :, :, HD : HD + 1], 1.0)

            # ---------------- Phase A: projections ----------------
            with (
                tc.tile_pool(name="wpool", bufs=1) as wpool,
                tc.tile_pool(name="panpool", bufs=2) as panpool,
                tc.tile_pool(name="pjpsum", bufs=4, space="PSUM") as pjpsum,
                tc.tile_pool(name="vpsum", bufs=4, space="PSUM") as vpsum,
            ):
                wq_sb = wpool.tile([P, KO, DL], BF16, tag="wq")
                wk_sb = wpool.tile([P, KO, DL], BF16, tag="wk")
                wv_sb = wpool.tile([P, KO, DL], BF16, tag="wv")
                for w_sb, W in ((wq_sb, WQT), (wk_sb, WKT), (wv_sb, WVT)):
                    nc.sync.dma_start(
                        out=w_sb[:], in_=W[:].rearrange("(ko p) m -> p ko m", p=P)
                    )

                for pan in range(NPAN):
                    qs = slice(pan * PANW, (pan + 1) * PANW)
                    qt_pan = panpool.tile([P, KO, PANW], BF16, tag="qt_pan")
                    kt_pan = panpool.tile([P, KO, PANW], BF16, tag="kt_pan")
                    for ko in range(KO):
                        nc.sync.dma_start(out=qt_pan[:, ko, :], in_=qt_r[:, ko, qs])
                        nc.sync.dma_start(out=kt_pan[:, ko, :], in_=kt_r[:, ko, qs])

                    # Q_^T and K_^T (feature-on-partition), bias fused in evict
                    for pan_in, w_sb, b_sb, dst in (
                        (qt_pan, wq_sb, bq_sb, qT),
                        (kt_pan, wk_sb, bk_sb, kT),
                    ):
                        for dc in range(2):
                            ps = pjpsum.tile([P, PANW], F32, tag="pj")
                            for ko in range(KO):
                                nc.tensor.matmul(
                                    ps[:],
                                    lhsT=(w_sb[:, ko, dc * P : (dc + 1) * P]),
                                    rhs=(pan_in[:, ko, :]),
                                    start=(ko == 0),
                                    stop=(ko == KO - 1),
                                )
                            nc.scalar.activation(
                                out=dst[:, dc, qs],
                                in_=ps[:],
                                func=AF.Identity,
                                bias=b_sb[:, dc : dc + 1],
                                scale=1.0,
                            )

                    # V natural layout (token-on-partition), bias via DVE add
                    for t4 in range(PANW // P):
                        tci = pan * (PANW // P) + t4
                        psv = vpsum.tile([P, DL], F32, tag="pv")
                        for ko in range(KO):
                            nc.tensor.matmul(
                                psv[:],
                                lhsT=(
                                    kt_pan[:, ko, t4 * P : (t4 + 1) * P]
                                ),
                                rhs=(wv_sb[:, ko, :]),
                                start=(ko == 0),
                                stop=(ko == KO - 1),
                            )
                        nc.vector.tensor_add(
                            out=v_sb[:, tci, :, 0:HD],
                            in0=psv[:].rearrange("p (h d) -> p h d", h=HL),
                            in1=bv_rep[:],
                        )

                # mask load last so it fills DMA gaps during phase A
                for kc in range(NKC):
                    nc.sync.dma_start(out=mT[:, kc, :], in_=mt_r[:, kc, :])

            # ---------------- Phase B: attention ----------------
            oT = otpool.tile([P, 2, N], BF16)
            with (
                tc.tile_pool(name="expool", bufs=2) as expool,
                tc.tile_pool(name="srpool", bufs=2) as srpool,
                tc.tile_pool(name="spsum", bufs=2, space="PSUM") as spsum,
                tc.tile_pool(name="opsum", bufs=2, space="PSUM") as opsum,
            ):
                for h in range(HL):
                    dc, po = h // 2, (h % 2) * HD
                    for qg in range(N // 1024):
                        ex = expool.tile([P, NKC, 1024], BF16, tag="ex")
                        for kc in range(NKC):
                            ps = spsum.tile([P, 1024], F32, tag="es")
                            for half in range(2):
                                q0 = qg * 1024 + half * 512
                                nc.tensor.matmul(
                                    ps[:, half * 512 : (half + 1) * 512],
                                    lhsT=kT[po : po + HD, dc, kc * P : (kc + 1) * P],
                                    rhs=qT[po : po + HD, dc, q0 : q0 + 512],
                                    start=True,
                                    stop=True,
                                )
                            nc.scalar.activation(
                                out=ex[:, kc, :], in_=ps[:], func=AF.Exp, scale=SCALE
                            )
                            nc.vector.tensor_mul(
                                out=ex[:, kc, :],
                                in0=ex[:, kc, :],
                                in1=mT[:, kc, qg * 1024 : (qg + 1) * 1024],
                            )
                        for qbh in range(2):
                            pso = opsum.tile([HD + 1, 512], F32, tag="pvo")
                            for kc in range(NKC):
                                nc.tensor.matmul(
                                    pso[:],
                                    lhsT=v_sb[:, kc, h, :],
                                    rhs=ex[:, kc, qbh * 512 : (qbh + 1) * 512],
                                    start=(kc == 0),
                                    stop=(kc == NKC - 1),
                                )
                            s_row = srpool.tile([1, 512], F32, tag="srow")
                            nc.scalar.copy(out=s_row[:], in_=pso[HD : HD + 1, :])
                            srp = opsum.tile([HD, 512], F32, tag="srp")
                            nc.tensor.matmul(
                                srp[:],
                                lhsT=ones_sb[:],
                                rhs=s_row[:],
                                start=True,
                                stop=True,
                            )
                            s_rep = srpool.tile([HD, 512], F32, tag="srep")
                            nc.vector.reciprocal(out=s_rep[:], in_=srp[:])
                            o_tmp = srpool.tile([HD, 512], BF16, tag="otmp")
                            nc.vector.tensor_mul(
                                out=o_tmp[:], in0=pso[0:HD, :], in1=s_rep[:]
                            )
                            q0 = qg * 1024 + qbh * 512
                            nc.sync.dma_start(
                                out=oT[po : po + HD, dc, q0 : q0 + 512], in_=o_tmp[:]
                            )

            # ---------------- Phase C: output projection ----------------
            with (
                tc.tile_pool(name="cout", bufs=3) as cout,
                tc.tile_pool(name="cpsum", bufs=4, space="PSUM") as cpsum,
            ):
                for qc in range(NQC):
                    pss = [
                        cpsum.tile([P, 512], F32, tag="co", name=f"co{i}")
                        for i in range(2)
                    ]
                    for cc in range(2):
                        for nh in range(2):
                            nc.tensor.matmul(
                                pss[nh][:],
                                lhsT=(oT[:, cc, qc * P : (qc + 1) * P]),
                                rhs=(wo_sb[:, cc, nh * 512 : (nh + 1) * 512]),
                                start=(cc == 0),
                                stop=(cc == 1),
                            )
                    o_sb = cout.tile([P, D], F32, tag="osb")
                    for nh in range(2):
                        nc.vector.tensor_copy(
                            out=o_sb[:, nh * 512 : (nh + 1) * 512], in_=pss[nh][:]
                        )
                    nc.sync.dma_start(out=OUT[qc * P : (qc + 1) * P, :], in_=o_sb[:])

    nc.finalize()
    return nc


_NC = None


def _get_nc():
    global _NC
    if _NC is None:
        _NC = build_nc()
    return _NC


def make_in_maps(Q, K, mask, Wq, bq, Wk, bk, Wv, bv, Wo, bo):
    Q = np.asarray(Q, np.float32)
    K = np.asarray(K, np.float32)
    mask = np.asarray(mask)
    Wq = np.asarray(Wq, np.float32)
    Wk = np.asarray(Wk, np.float32)
    Wv = np.asarray(Wv, np.float32)
    Wo = np.asarray(Wo, np.float32)
    qt = [np.ascontiguousarray(Q[b].T).astype(ml_dtypes.bfloat16) for b in range(B)]
    kt = [np.ascontiguousarray(K[b].T).astype(ml_dtypes.bfloat16) for b in range(B)]
    mt = [
        np.ascontiguousarray(mask[b].T).astype(ml_dtypes.bfloat16) for b in range(B)
    ]
    in_maps = []
    for c in range(8):
        b, hg = divmod(c, 4)
        cols = slice(hg * DL, (hg + 1) * DL)
        in_maps.append(
            {
                "qt": qt[b],
                "kt": kt[b],
                "mt": mt[b],
                "wqt": np.ascontiguousarray(Wq[cols, :].T).astype(ml_dtypes.bfloat16),
                "wkt": np.ascontiguousarray(Wk[cols, :].T).astype(ml_dtypes.bfloat16),
                "wvt": np.ascontiguousarray(Wv[cols, :].T).astype(ml_dtypes.bfloat16),
                "wot": np.ascontiguousarray(Wo[:, cols].T).astype(ml_dtypes.bfloat16),
                "bq": np.ascontiguousarray(np.asarray(bq, np.float32)[cols]),
                "bk": np.ascontiguousarray(np.asarray(bk, np.float32)[cols]),
                "bv": np.ascontiguousarray(np.asarray(bv, np.float32)[cols]),
            }
        )
    return in_maps


def assemble(results, bo):
    O = np.zeros((B, N, D), np.float32)
    for c in range(8):
        b = c // 4
        O[b] += results[c]["out"]
    O += np.asarray(bo, np.float32)[None, None, :]
    return O


def kernel(Q, K, mask, Wq, bq, Wk, bk, Wv, bv, Wo, bo):
    nc = _get_nc()
    in_maps = make_in_maps(Q, K, mask, Wq, bq, Wk, bk, Wv, bv, Wo, bo)
    res = run_bass_kernel_spmd(nc, in_maps, core_ids=list(range(8)))
    return assemble(res.results, bo)



# revision 30
# speedup vs baseline: 1.1562x; 1.1562x over previous
"""Trainium2 Bass kernel for nn_MHA_36584531427723.

Sharding: 8 cores = 2 batches x 4 head-groups (4 heads of 64 dims per core).
Each core: Q/K/V projections for its 256 features, attention for its 4 heads,
partial output projection (its 256 rows of Wo^T). Host sums 4 partials + bo.

Single fused pipeline per core (vs the phase-sequential baseline):
  - Scores computed transposed E^T[k,q] (contraction=head_dim on partitions).
  - exp on the Act engine in 1024-wide tiles; it is the near-critical path, so
    the PE stream is emitted as E-matmuls interleaved with small "filler"
    units (projections, PV, transpose, C) kept under ~450ns each so the exp
    stream never starves; a budget scheduler pops fillers between E-matmuls.
  - PV in [q, 65] orientation (out free = 65 -> half the PE cost of the
    baseline's [65, q] form); a ones column in V gives softmax denominators.
  - o normalized per-q via DVE tensor_scalar with per-partition reciprocal,
    PE-transposed (bf16) to feature-major for the C matmul.
  - masked softmax: exp then tensor_tensor multiply with bf16 0/1 mask
    (DVE 2x_1p; 1/4 of tiles on GpSimd); max-subtraction and +eps dropped
    (|E|~<2; relative effect ~1e-11, same argument as the baseline).
  - OUT written bf16 (host accumulates partials in f32).
"""

import numpy as np
import ml_dtypes

import concourse.bacc as bacc
import concourse.bass as bass  # noqa: F401
import concourse.mybir as mybir
import concourse.tile as tile
from concourse.bass_utils import run_bass_kernel_spmd

B, N, D = 2, 2048, 1024
H = 16
HD = 64
HL = 4  # heads per core
DL = HL * HD  # 256 local features
P = 128
KO = D // P  # 8 contraction chunks for projections
NKC = N // P  # 16 k-token chunks
PAN = 1024  # q panel width in phase B
NPAN = N // PAN  # 2
QCP = PAN // P  # 8 q-chunks per panel
SCALE = 1.0 / 32.0  # 1/sqrt(DIM_V)

F32 = mybir.dt.float32
BF16 = mybir.dt.bfloat16
AF = mybir.ActivationFunctionType


def build_nc():
    nc = bacc.Bacc(None, target_bir_lowering=False)
    QT = nc.dram_tensor("qt", (D, N), BF16, kind="ExternalInput")
    KT = nc.dram_tensor("kt", (D, N), BF16, kind="ExternalInput")
    # mask, transposed + panel-major: [pan, p, kc, q'] = mask[b].T[kc*128+p, pan*1024+q']
    MT = nc.dram_tensor("mt", (NPAN, P, NKC, PAN), BF16, kind="ExternalInput")
    WQT = nc.dram_tensor("wqt", (D, DL), BF16, kind="ExternalInput")
    WKT = nc.dram_tensor("wkt", (D, DL), BF16, kind="ExternalInput")
    WVT = nc.dram_tensor("wvt", (D, DL), BF16, kind="ExternalInput")
    WOT = nc.dram_tensor("wot", (DL, D), BF16, kind="ExternalInput")
    BQ = nc.dram_tensor("bq", (DL,), F32, kind="ExternalInput")
    BK = nc.dram_tensor("bk", (DL,), F32, kind="ExternalInput")
    BV = nc.dram_tensor("bv", (DL,), F32, kind="ExternalInput")
    IDENT = nc.dram_tensor("ident", (P, P), BF16, kind="ExternalInput")
    OUT = nc.dram_tensor("out", (N, D), BF16, kind="ExternalOutput")

    qt_r = QT[:].rearrange("(ko p) q -> p ko q", p=P)
    kt_r = KT[:].rearrange("(ko p) q -> p ko q", p=P)

    uid = [0]

    def nm(pfx):
        uid[0] += 1
        return f"{pfx}{uid[0]}"

    with tile.TileContext(nc) as tc:
        with (
            tc.tile_pool(name="persist", bufs=1) as persist,
            tc.tile_pool(name="panpool", bufs=3) as panpool,
            tc.tile_pool(name="mpool", bufs=2) as mpool,
            tc.tile_pool(name="expool", bufs=2) as expool,
            tc.tile_pool(name="oqpool", bufs=2) as oqpool,
            tc.tile_pool(name="otpool", bufs=1) as otpool,
            tc.tile_pool(name="csb", bufs=2) as csb,
            tc.tile_pool(name="rcpool", bufs=2) as rcpool,
            tc.tile_pool(name="pepool", bufs=2, space="PSUM") as pepool,
            tc.tile_pool(name="cpool", bufs=2, space="PSUM") as cpool,
            tc.tile_pool(name="tpool", bufs=2, space="PSUM") as tpool,
        ):
            # ---- persistent SBUF ----
            qT = persist.tile([P, 2, N], BF16, tag="qT")
            kT = persist.tile([P, 2, N], BF16, tag="kT")
            v_sb = persist.tile([P, NKC, HL, HD + 1], BF16, tag="v")
            wq_sb = persist.tile([P, KO, DL], BF16, tag="wq")
            wk_sb = persist.tile([P, KO, DL], BF16, tag="wk")
            wv_sb = persist.tile([P, KO, DL], BF16, tag="wv")
            wo_sb = persist.tile([P, 2, D], BF16, tag="wo")
            bq_sb = persist.tile([P, 2], F32, tag="bq")
            bk_sb = persist.tile([P, 2], F32, tag="bk")
            bv_rep = persist.tile([P, HL, HD], F32, tag="bv")
            ident = persist.tile([P, P], BF16, tag="ident")

            m_half = {}

            def mask_dma(pan, half):
                """Mask half-panel [128, 8 kc, 1024]; two quad-DMAs."""
                m_half[(pan, half)] = mt = mpool.tile(
                    [P, NKC // 2, PAN], BF16, tag="m", name=f"m{pan}_{half}"
                )
                for qd in range(2):
                    nc.sync.dma_start(
                        out=mt[:, qd * 4 : (qd + 1) * 4, :],
                        in_=MT[pan, :, half * 8 + qd * 4 : half * 8 + (qd + 1) * 4, :],
                    )

            # ---- startup DMAs, ordered for earliest gapless exp stream ----
            def half_dma(t, src_r, col0, s):
                nc.sync.dma_start(
                    out=t[:, :, s * 512 : (s + 1) * 512],
                    in_=src_r[:, :, col0 + s * 512 : col0 + (s + 1) * 512],
                )

            nc.sync.dma_start(
                out=wk_sb[:], in_=WKT[:].rearrange("(ko p) m -> p ko m", p=P)
            )
            ktA = panpool.tile([P, KO, PAN], BF16, tag="pan", name="ktA")
            half_dma(ktA, kt_r, 0, 0)
            nc.sync.dma_start(
                out=wq_sb[:], in_=WQT[:].rearrange("(ko p) m -> p ko m", p=P)
            )
            qt0 = panpool.tile([P, KO, PAN], BF16, tag="pan", name="qt0")
            half_dma(qt0, qt_r, 0, 0)
            half_dma(qt0, qt_r, 0, 1)
            nc.sync.dma_start(out=bk_sb[:], in_=BK[:].rearrange("(c p) -> p c", p=P))
            nc.sync.dma_start(out=bq_sb[:], in_=BQ[:].rearrange("(c p) -> p c", p=P))
            half_dma(ktA, kt_r, 0, 1)
            nc.sync.dma_start(
                out=wv_sb[:], in_=WVT[:].rearrange("(ko p) m -> p ko m", p=P)
            )
            nc.sync.dma_start(
                out=bv_rep[:],
                in_=BV[:].rearrange("(h d) -> h d", h=HL)[None].to_broadcast(
                    (P, HL, HD)
                ),
            )
            ktB = panpool.tile([P, KO, PAN], BF16, tag="pan", name="ktB")
            half_dma(ktB, kt_r, PAN, 0)
            half_dma(ktB, kt_r, PAN, 1)
            mask_dma(0, 0)  # 2 quad DMAs
            mask_dma(0, 1)
            nc.vector.memset(v_sb[:, :, :, HD : HD + 1], 1.0)

            def late_dmas():
                nc.sync.dma_start(
                    out=wo_sb[:], in_=WOT[:].rearrange("(cc p) n -> p cc n", p=P)
                )
                nc.sync.dma_start(out=ident[:], in_=IDENT[:])

            # ---- helper emitters ----
            def proj_quarters(dst, fslice, w_sb, bias_ap, pan_cell, panslice):
                """One [128 x 512] projection as a list of 4 filler units
                (2 ko-steps each, ~426ns PE) sharing one cpool tile.
                pan_cell: 1-elem list deref'd lazily (reload tiles)."""
                cell = [None]

                def q_unit(qi):
                    if qi == 0:
                        cell[0] = cpool.tile([P, 512], F32, tag="c", name=nm("pq"))
                    ps = cell[0]
                    for ko in range(2 * qi, 2 * qi + 2):
                        nc.tensor.matmul(
                            ps[:],
                            lhsT=w_sb[:, ko, fslice],
                            rhs=pan_cell[0][:, ko, panslice],
                            start=(ko == 0),
                            stop=(ko == KO - 1),
                        )
                    if qi == 3:
                        nc.vector.tensor_scalar_add(dst, ps[:], bias_ap)

                return [(426, lambda qi=qi: q_unit(qi)) for qi in range(4)]

            def vproj_halves(pan_cell, t4, kc):
                """V for one 128-token chunk (all 4 heads) as 2 filler units."""
                cell = [None]

                def h_unit(hi):
                    if hi == 0:
                        cell[0] = cpool.tile([P, 512], F32, tag="c", name=nm("pq"))
                    ps = cell[0]
                    for ko in range(4 * hi, 4 * hi + 4):
                        nc.tensor.matmul(
                            ps[:, 0:DL],
                            lhsT=pan_cell[0][:, ko, t4 * P : (t4 + 1) * P],
                            rhs=wv_sb[:, ko, :],
                            start=(ko == 0),
                            stop=(ko == KO - 1),
                        )
                    if hi == 1:
                        nc.vector.tensor_add(
                            out=v_sb[:, kc, :, 0:HD],
                            in0=ps[:, 0:DL].rearrange("p (h d) -> p h d", h=HL),
                            in1=bv_rep[:],
                        )

                return [(428, lambda hi=hi: h_unit(hi)) for hi in range(2)]

            def pv_norm_unit(h, pan, qc, ex, o_qd):
                """PV for one 128-q chunk + normalize into o_qd (bf16)."""
                ps = cpool.tile([P, 512], F32, tag="c")
                pso = ps[:, 0 : HD + 1]
                for kc in range(NKC):
                    nc.tensor.matmul(
                        pso,
                        lhsT=ex[:, kc, qc * P : (qc + 1) * P],
                        rhs=v_sb[:, kc, h, :],
                        start=(kc == 0),
                        stop=(kc == NKC - 1),
                    )
                rcp = rcpool.tile([P, 1], F32, tag="rcp")
                nc.vector.reciprocal(out=rcp[:], in_=pso[:, HD : HD + 1])
                nc.vector.tensor_scalar(
                    out=o_qd[:, qc, h * HD : (h + 1) * HD],
                    in0=pso[:, 0:HD],
                    scalar1=rcp[:],
                    scalar2=None,
                    op0=mybir.AluOpType.mult,
                )

            def tc_units(pan, qc, o_qd, oT_pan, tail):
                """Transpose + C for one q-chunk, as 3 filler units.
                In the tail, one C eviction moves to the otherwise-idle Act."""
                dve_evict = lambda out, in_: nc.vector.tensor_copy(out=out, in_=in_)

                def t_unit():
                    for fc in range(2):
                        tp = tpool.tile([P, P], BF16, tag="t")
                        nc.tensor.transpose(
                            tp[:], o_qd[:, qc, fc * P : (fc + 1) * P], ident[:]
                        )
                        dve_evict(out=oT_pan[:, fc, qc * P : (qc + 1) * P], in_=tp[:])

                cell = [None]

                def c_unit(half):
                    if half == 0:
                        cell[0] = csb.tile([P, 2, 512], BF16, tag="co", name=nm("co"))
                    cp = cpool.tile([P, 512], F32, tag="c")
                    for fc in range(2):
                        nc.tensor.matmul(
                            cp[:],
                            lhsT=oT_pan[:, fc, qc * P : (qc + 1) * P],
                            rhs=wo_sb[:, fc, half * 512 : (half + 1) * 512],
                            start=(fc == 0),
                            stop=(fc == 1),
                        )
                    if tail and half == 1:
                        nc.scalar.copy(out=cell[0][:, half, :], in_=cp[:])
                    else:
                        dve_evict(out=cell[0][:, half, :], in_=cp[:])
                    if half == 1:
                        q0 = pan * PAN + qc * P
                        nc.sync.dma_start(out=OUT[q0 : q0 + P, :], in_=cell[0][:])

                return [(120, t_unit), (426, lambda: c_unit(0)), (426, lambda: c_unit(1))]

            # ---- startup projections: K(dc0) tokens 0..511, Q(p0,dc0) ----
            for est, f in proj_quarters(
                kT[:, 0, 0:512], slice(0, P), wk_sb,
                bk_sb[:, 0:1], [ktA], slice(0, 512),
            ):
                f()
            for u in range(2):
                for est, f in proj_quarters(
                    qT[:, 0, u * 512 : (u + 1) * 512], slice(0, P), wq_sb,
                    bq_sb[:, 0:1], [qt0], slice(u * 512, (u + 1) * 512),
                ):
                    f()

            # K(dc0) for tokens 1024..2047 trails into the B loop as fillers
            fillers = []
            est_state = {"pe": 18000.0, "act": 0.0, "hist": []}

            def addf(units, key=None):
                for est, f in units:
                    fillers.append((est, key, f))

            addf(
                proj_quarters(
                    kT[:, 0, 512:1024], slice(0, P),
                    wk_sb, bk_sb[:, 0:1], [ktA], slice(512, 1024),
                ),
                key="kdc0",
            )
            addf(
                proj_quarters(
                    kT[:, 0, PAN : PAN + 512], slice(0, P),
                    wk_sb, bk_sb[:, 0:1], [ktB], slice(0, 512),
                ),
                key="kdc0",
            )
            # Q(p0, dc1)
            for u in range(2):
                addf(
                    proj_quarters(
                        qT[:, 1, u * 512 : (u + 1) * 512], slice(P, 2 * P), wq_sb,
                        bq_sb[:, 1:2], [qt0], slice(u * 512, (u + 1) * 512),
                    ),
                    key="qp0",
                )
            # V + K(dc1) straight from the resident ktA/ktB panels
            for t4 in range(8):
                addf(vproj_halves([ktA], t4, t4), key="vk")
                if t4 == 1:
                    # second half of K(dc0) (needs the later ktB half-DMA)
                    addf(
                        proj_quarters(
                            kT[:, 0, PAN + 512 : N], slice(0, P),
                            wk_sb, bk_sb[:, 0:1], [ktB], slice(512, 1024),
                        ),
                        key="kdc0",
                    )
                if t4 == 3:
                    addf(
                        proj_quarters(
                            kT[:, 1, 0:512], slice(P, 2 * P), wk_sb,
                            bk_sb[:, 1:2], [ktA], slice(0, 512),
                        ),
                        key="vk",
                    )
            addf(
                proj_quarters(
                    kT[:, 1, 512:1024], slice(P, 2 * P), wk_sb, bk_sb[:, 1:2],
                    [ktA], slice(512, 1024),
                ),
                key="vk",
            )
            for t4 in range(8):
                addf(vproj_halves([ktB], t4, 8 + t4), key="vkb")
                if t4 == 3:
                    addf(
                        proj_quarters(
                            kT[:, 1, PAN : PAN + 512], slice(P, 2 * P), wk_sb,
                            bk_sb[:, 1:2], [ktB], slice(0, 512),
                        ),
                        key="vkb",
                    )
            addf(
                proj_quarters(
                    kT[:, 1, PAN + 512 : N], slice(P, 2 * P), wk_sb, bk_sb[:, 1:2],
                    [ktB], slice(512, 1024),
                ),
                key="vkb",
            )
            # qt panel 1 (4th 'pan' ring alloc frees ktA) + Q(p1) both dc
            qt1 = [None]

            def alloc_dma_qt1():
                qt1[0] = t = panpool.tile([P, KO, PAN], BF16, tag="pan", name="qt1")
                for s in range(2):
                    nc.sync.dma_start(
                        out=t[:, :, s * 512 : (s + 1) * 512],
                        in_=qt_r[:, :, PAN + s * 512 : PAN + (s + 1) * 512],
                    )

            addf([(0, alloc_dma_qt1)], key="qp1")
            for dc in range(2):
                for u in range(2):
                    addf(
                        proj_quarters(
                            qT[:, dc, PAN + u * 512 : PAN + (u + 1) * 512],
                            slice(dc * P, (dc + 1) * P), wq_sb,
                            bq_sb[:, dc : dc + 1], qt1, slice(u * 512, (u + 1) * 512),
                        ),
                        key="qp1",
                    )
            addf([(0, late_dmas)])

            def emit_fillers():
                """Pop fillers while the estimated PE clock trails the
                estimated Act clock (keeps exp gapless without bursting)."""
                while fillers:
                    est, key, f = fillers[0]
                    if est_state["pe"] + est > est_state["act"] - 430:
                        break
                    fillers.pop(0)
                    f()
                    est_state["pe"] += est

            def drain_through(keys):
                """Force-emit queue entries up to the last entry whose key is
                in `keys` (emission-order deadlines at head boundaries)."""
                if not any(k in keys for _, k, _ in fillers):
                    return
                last = max(i for i, (_, k, _) in enumerate(fillers) if k in keys)
                for est, _, f in fillers[: last + 1]:
                    f()
                    est_state["pe"] += est
                del fillers[: last + 1]

            # ---- fused B/C pipeline ----
            pending = []
            for pan in range(NPAN):
                o_qd = oqpool.tile([P, QCP, DL], BF16, tag="oq", name=f"oq{pan}")
                oT_pan = otpool.tile([P, 2, PAN], BF16, tag="ot", name=f"ot{pan}")
                for h in range(HL):
                    g = pan * HL + h
                    dc, po = h // 2, (h % 2) * HD
                    # emission-order deadlines (before this head's first E):
                    # kT(dc1) writers before h2; qT(p1) before panel 1.
                    need = set()
                    if pan == 0 and h == 2:
                        need.add("vk")
                    if pan == 1 and h == 0:
                        need.add("qp1")
                    drain_through(need)
                    deferred_vkb = pan == 0 and h == 2
                    ex = expool.tile([P, NKC, PAN], BF16, tag="ex", name=f"ex{pan}_{h}")
                    for kc in range(NKC):
                        ps = pepool.tile([P, PAN], F32, tag="e")
                        for eh in range(2):
                            nc.tensor.matmul(
                                ps[:, eh * 512 : (eh + 1) * 512],
                                lhsT=kT[po : po + HD, dc, kc * P : (kc + 1) * P],
                                rhs=qT[
                                    po : po + HD,
                                    dc,
                                    pan * PAN + eh * 512 : pan * PAN + (eh + 1) * 512,
                                ],
                                start=True,
                                stop=True,
                            )
                        hist = est_state["hist"]
                        if len(hist) >= 2:
                            est_state["pe"] = max(est_state["pe"], hist[-2] + 250)
                        est_state["pe"] += 427
                        nc.scalar.activation(
                            out=ex[:, kc, :], in_=ps[:], func=AF.Exp, scale=SCALE
                        )
                        prev = hist[-1] if hist else est_state["pe"] + 870
                        hist.append(max(prev, est_state["pe"] + 250) + 1038)
                        est_state["act"] = hist[-1]
                        if kc == 3 and pending:
                            vk_idx = [
                                i
                                for i, (_, k, _) in enumerate(fillers)
                                if k in ("vk", "vkb")
                            ]
                            at = (vk_idx[-1] + 1) if vk_idx else 0
                            fillers[at:at] = pending
                            pending.clear()
                        if kc == 2 and g >= 2:
                            drain_through({("pv", g - 2)})
                        if pan == 0 and h == 3 and kc == 2:
                            drain_through({"vk23"})
                        if deferred_vkb and kc == 5:
                            drain_through({"vkb"})
                        emit_fillers()
                        last_head = pan == NPAN - 1 and h == HL - 1
                        if last_head and kc >= 14:
                            mt = m_half[(pan, 1)]
                            k0 = kc - 8
                            nc.vector.tensor_mul(
                                out=ex[:, kc : kc + 1, :],
                                in0=ex[:, kc : kc + 1, :],
                                in1=mt[:, k0 : k0 + 1, :],
                            )
                        elif kc % 2 == 1:
                            pair = (kc - 1) // 2
                            mt = m_half[(pan, kc // 8)]
                            k0 = kc - 1 - (kc // 8) * 8
                            nc.vector.tensor_mul(
                                out=ex[:, kc - 1 : kc + 1, :],
                                in0=ex[:, kc - 1 : kc + 1, :],
                                in1=mt[:, k0 : k0 + 2, :],
                            )
                        if pan == 0 and h == 3 and kc == 2:
                            mask_dma(1, 0)
                        if pan == 0 and h == 3 and kc == 10:
                            mask_dma(1, 1)
                    # queue this head's PV into `pending`: released at kc==3
                    # of the next head, after that head's last masks have had
                    # time to finish (PV's kc14/15 matmuls depend on them)
                    for qc in range(QCP):
                        pending.append(
                            (
                                432,
                                ("pv", g),
                                lambda h=h, pan=pan, qc=qc, ex=ex, o_qd=o_qd: pv_norm_unit(
                                    h, pan, qc, ex, o_qd
                                ),
                            )
                        )
                    if h == HL - 1:
                        for qc in range(QCP):
                            for est, f in tc_units(
                                pan, qc, o_qd, oT_pan, tail=(pan == NPAN - 1)
                            ):
                                pending.append((est, "tc", f))

            # drain remaining fillers; software-pipeline the last panel's
            # PV -> T -> C chains (PV j+1 before T j, C j-1 after T j)
            while fillers:
                fillers.pop(0)[2]()
            pvs = [f for _, k, f in pending if k != "tc"]
            tcs = [f for _, k, f in pending if k == "tc"]
            tcg = [tcs[i * 3 : (i + 1) * 3] for i in range(len(tcs) // 3)]
            pending.clear()
            stages = []
            for j in range(len(pvs)):
                stages.append(pvs[j])
                if j >= 1 and j - 1 < len(tcg):
                    stages.extend(tcg[j - 1][:1])  # transpose of j-1
                if j >= 2 and j - 2 < len(tcg):
                    stages.extend(tcg[j - 2][1:])  # C halves of j-2
            for j in range(max(0, len(pvs) - 1), len(tcg)):
                stages.extend(tcg[j][:1])
            for j in range(max(0, len(pvs) - 2), len(tcg)):
                stages.extend(tcg[j][1:])
            for f in stages:
                f()

    nc.finalize()
    return nc


_NC = None


def _get_nc():
    global _NC
    if _NC is None:
        _NC = build_nc()
    return _NC


def make_in_maps(Q, K, mask, Wq, bq, Wk, bk, Wv, bv, Wo, bo):
    Q = np.asarray(Q, np.float32)
    K = np.asarray(K, np.float32)
    mask = np.asarray(mask)
    Wq = np.asarray(Wq, np.float32)
    Wk = np.asarray(Wk, np.float32)
    Wv = np.asarray(Wv, np.float32)
    Wo = np.asarray(Wo, np.float32)
    qt = [np.ascontiguousarray(Q[b].T).astype(ml_dtypes.bfloat16) for b in range(B)]
    kt = [np.ascontiguousarray(K[b].T).astype(ml_dtypes.bfloat16) for b in range(B)]
    # [pan, p, kc, q'] = mask[b].T[kc*128+p, pan*1024+q']
    mt = [
        np.ascontiguousarray(
            mask[b].T.reshape(NKC, P, NPAN, PAN).transpose(2, 1, 0, 3)
        ).astype(ml_dtypes.bfloat16)
        for b in range(B)
    ]
    ident = np.eye(P, dtype=np.float32).astype(ml_dtypes.bfloat16)
    in_maps = []
    for c in range(8):
        b, hg = divmod(c, 4)
        cols = slice(hg * DL, (hg + 1) * DL)
        in_maps.append(
            {
                "qt": qt[b],
                "kt": kt[b],
                "mt": mt[b],
                "wqt": np.ascontiguousarray(Wq[cols, :].T).astype(ml_dtypes.bfloat16),
                "wkt": np.ascontiguousarray(Wk[cols, :].T).astype(ml_dtypes.bfloat16),
                "wvt": np.ascontiguousarray(Wv[cols, :].T).astype(ml_dtypes.bfloat16),
                "wot": np.ascontiguousarray(Wo[:, cols].T).astype(ml_dtypes.bfloat16),
                "bq": np.ascontiguousarray(np.asarray(bq, np.float32)[cols]),
                "bk": np.ascontiguousarray(np.asarray(bk, np.float32)[cols]),
                "bv": np.ascontiguousarray(np.asarray(bv, np.float32)[cols]),
                "ident": ident,
            }
        )
    return in_maps


def assemble(results, bo):
    O = np.zeros((B, N, D), np.float32)
    for c in range(8):
        b = c // 4
        O[b] += np.asarray(results[c]["out"], np.float32)
    O += np.asarray(bo, np.float32)[None, None, :]
    return O


def kernel(Q, K, mask, Wq, bq, Wk, bk, Wv, bv, Wo, bo):
    nc = _get_nc()
    in_maps = make_in_maps(Q, K, mask, Wq, bq, Wk, bk, Wv, bv, Wo, bo)
    res = run_bass_kernel_spmd(nc, in_maps, core_ids=list(range(8)))
    return assemble(res.results, bo)


# revision 37
# speedup vs baseline: 1.2437x; 1.0757x over previous
"""Trainium2 Bass kernel for nn_MHA_36584531427723.

Sharding: 8 cores = 2 batches x 4 head-groups (4 heads of 64 dims per core).
Each core: Q/K/V projections for its 256 features, attention for its 4 heads,
partial output projection (its 256 rows of Wo^T). Host sums 4 partials + bo.

Single fused pipeline per core (vs the phase-sequential baseline):
  - Scores computed transposed E^T[k,q] (contraction=head_dim on partitions).
  - exp on the Act engine in 1024-wide tiles; it is the near-critical path, so
    the PE stream is emitted as E-matmuls interleaved with small "filler"
    units (projections, PV, transpose, C) kept under ~450ns each so the exp
    stream never starves; a budget scheduler pops fillers between E-matmuls.
  - PV in [q, 65] orientation (out free = 65 -> half the PE cost of the
    baseline's [65, q] form); a ones column in V gives softmax denominators.
  - o normalized per-q via DVE tensor_scalar with per-partition reciprocal,
    PE-transposed (bf16) to feature-major for the C matmul.
  - masked softmax: exp then tensor_tensor multiply with bf16 0/1 mask
    (DVE 2x_1p; 1/4 of tiles on GpSimd); max-subtraction and +eps dropped
    (|E|~<2; relative effect ~1e-11, same argument as the baseline).
  - OUT written bf16 (host accumulates partials in f32).
"""

import numpy as np
import ml_dtypes

import concourse.bacc as bacc
import concourse.bass as bass  # noqa: F401
import concourse.mybir as mybir
import concourse.tile as tile
from concourse.bass_utils import run_bass_kernel_spmd

B, N, D = 2, 2048, 1024
H = 16
HD = 64
HL = 4  # heads per core
DL = HL * HD  # 256 local features
P = 128
KO = D // P  # 8 contraction chunks for projections
NKC = N // P  # 16 k-token chunks
PAN = 1024  # q panel width in phase B
NPAN = N // PAN  # 2
QCP = PAN // P  # 8 q-chunks per panel
SCALE = 1.0 / 32.0  # 1/sqrt(DIM_V)

F32 = mybir.dt.float32
BF16 = mybir.dt.bfloat16
AF = mybir.ActivationFunctionType


def build_nc():
    nc = bacc.Bacc(None, target_bir_lowering=False)
    QT = nc.dram_tensor("qt", (D, N), BF16, kind="ExternalInput")
    KT = nc.dram_tensor("kt", (D, N), BF16, kind="ExternalInput")
    # mask, transposed + panel-major: [pan, p, kc, q'] = mask[b].T[kc*128+p, pan*1024+q']
    MT = nc.dram_tensor("mt", (NPAN, P, NKC, PAN), BF16, kind="ExternalInput")
    WQT = nc.dram_tensor("wqt", (D, DL), BF16, kind="ExternalInput")
    WKT = nc.dram_tensor("wkt", (D, DL), BF16, kind="ExternalInput")
    WVT = nc.dram_tensor("wvt", (D, DL), BF16, kind="ExternalInput")
    WOT = nc.dram_tensor("wot", (DL, D), BF16, kind="ExternalInput")
    BQ = nc.dram_tensor("bq", (DL,), F32, kind="ExternalInput")
    BK = nc.dram_tensor("bk", (DL,), F32, kind="ExternalInput")
    BV = nc.dram_tensor("bv", (DL,), F32, kind="ExternalInput")
    IDENT = nc.dram_tensor("ident", (P, P), BF16, kind="ExternalInput")
    OUT = nc.dram_tensor("out", (N, D), BF16, kind="ExternalOutput")

    qt_r = QT[:].rearrange("(ko p) q -> p ko q", p=P)
    kt_r = KT[:].rearrange("(ko p) q -> p ko q", p=P)

    uid = [0]

    def nm(pfx):
        uid[0] += 1
        return f"{pfx}{uid[0]}"

    with tile.TileContext(nc) as tc:
        with (
            tc.tile_pool(name="persist", bufs=1) as persist,
            tc.tile_pool(name="panpool", bufs=3) as panpool,
            tc.tile_pool(name="mpool", bufs=2) as mpool,
            tc.tile_pool(name="expool", bufs=2) as expool,
            tc.tile_pool(name="oqpool", bufs=2) as oqpool,
            tc.tile_pool(name="otpool", bufs=1) as otpool,
            tc.tile_pool(name="csb", bufs=2) as csb,
            tc.tile_pool(name="rcpool", bufs=2) as rcpool,
            tc.tile_pool(name="pepool", bufs=2, space="PSUM") as pepool,
            tc.tile_pool(name="cpool", bufs=2, space="PSUM") as cpool,
            tc.tile_pool(name="tpool", bufs=2, space="PSUM") as tpool,
        ):
            # ---- persistent SBUF ----
            qT = persist.tile([P, 2, N], BF16, tag="qT")
            kT = persist.tile([P, 2, N], BF16, tag="kT")
            v_sb = persist.tile([P, NKC, HL, HD + 1], BF16, tag="v")
            wq_sb = persist.tile([P, KO, DL], BF16, tag="wq")
            wk_sb = persist.tile([P, KO, DL], BF16, tag="wk")
            wv_sb = persist.tile([P, KO, DL], BF16, tag="wv")
            wo_sb = persist.tile([P, 2, D], BF16, tag="wo")
            bq_sb = persist.tile([P, 2], F32, tag="bq")
            bk_sb = persist.tile([P, 2], F32, tag="bk")
            bv_rep = persist.tile([P, HL, HD], F32, tag="bv")
            ident = persist.tile([P, P], BF16, tag="ident")

            m_half = {}

            def mask_dma(pan, half):
                """Mask half-panel [128, 8 kc, 1024]; two quad-DMAs."""
                m_half[(pan, half)] = mt = mpool.tile(
                    [P, NKC // 2, PAN], BF16, tag="m", name=f"m{pan}_{half}"
                )
                for qd in range(2):
                    nc.sync.dma_start(
                        out=mt[:, qd * 4 : (qd + 1) * 4, :],
                        in_=MT[pan, :, half * 8 + qd * 4 : half * 8 + (qd + 1) * 4, :],
                    )

            # ---- startup DMAs, ordered for earliest gapless exp stream ----
            def half_dma(t, src_r, col0, s):
                nc.sync.dma_start(
                    out=t[:, :, s * 512 : (s + 1) * 512],
                    in_=src_r[:, :, col0 + s * 512 : col0 + (s + 1) * 512],
                )

            nc.sync.dma_start(
                out=wk_sb[:], in_=WKT[:].rearrange("(ko p) m -> p ko m", p=P)
            )
            ktA = panpool.tile([P, KO, PAN], BF16, tag="pan", name="ktA")
            half_dma(ktA, kt_r, 0, 0)
            nc.sync.dma_start(
                out=wq_sb[:], in_=WQT[:].rearrange("(ko p) m -> p ko m", p=P)
            )
            qt0 = panpool.tile([P, KO, PAN], BF16, tag="pan", name="qt0")
            half_dma(qt0, qt_r, 0, 0)
            half_dma(qt0, qt_r, 0, 1)
            nc.sync.dma_start(out=bk_sb[:], in_=BK[:].rearrange("(c p) -> p c", p=P))
            nc.sync.dma_start(out=bq_sb[:], in_=BQ[:].rearrange("(c p) -> p c", p=P))
            half_dma(ktA, kt_r, 0, 1)
            nc.sync.dma_start(
                out=wv_sb[:], in_=WVT[:].rearrange("(ko p) m -> p ko m", p=P)
            )
            nc.sync.dma_start(
                out=bv_rep[:],
                in_=BV[:].rearrange("(h d) -> h d", h=HL)[None].to_broadcast(
                    (P, HL, HD)
                ),
            )
            ktB = panpool.tile([P, KO, PAN], BF16, tag="pan", name="ktB")
            half_dma(ktB, kt_r, PAN, 0)
            half_dma(ktB, kt_r, PAN, 1)
            mask_dma(0, 0)  # 2 quad DMAs
            mask_dma(0, 1)
            nc.vector.memset(v_sb[:, :, :, HD : HD + 1], 1.0)

            def late_dmas():
                nc.sync.dma_start(
                    out=wo_sb[:], in_=WOT[:].rearrange("(cc p) n -> p cc n", p=P)
                )
                nc.sync.dma_start(out=ident[:], in_=IDENT[:])

            # ---- helper emitters ----
            def proj_quarters(dst, fslice, w_sb, bias_ap, pan_cell, panslice):
                """One [128 x 512] projection as a list of 4 filler units
                (2 ko-steps each, ~426ns PE) sharing one cpool tile.
                pan_cell: 1-elem list deref'd lazily (reload tiles)."""
                cell = [None]

                def q_unit(qi):
                    if qi == 0:
                        cell[0] = cpool.tile([P, 512], F32, tag="c", name=nm("pq"))
                    ps = cell[0]
                    for ko in range(2 * qi, 2 * qi + 2):
                        nc.tensor.matmul(
                            ps[:],
                            lhsT=w_sb[:, ko, fslice],
                            rhs=pan_cell[0][:, ko, panslice],
                            start=(ko == 0),
                            stop=(ko == KO - 1),
                        )
                    if qi == 3:
                        nc.vector.tensor_scalar_add(dst, ps[:], bias_ap)

                return [(426, lambda qi=qi: q_unit(qi)) for qi in range(4)]

            def vproj_halves(pan_cell, t4, kc):
                """V for one 128-token chunk (all 4 heads) as 2 filler units."""
                cell = [None]

                def h_unit(hi):
                    if hi == 0:
                        cell[0] = cpool.tile([P, 512], F32, tag="c", name=nm("pq"))
                    ps = cell[0]
                    for ko in range(4 * hi, 4 * hi + 4):
                        nc.tensor.matmul(
                            ps[:, 0:DL],
                            lhsT=pan_cell[0][:, ko, t4 * P : (t4 + 1) * P],
                            rhs=wv_sb[:, ko, :],
                            start=(ko == 0),
                            stop=(ko == KO - 1),
                        )
                    if hi == 1:
                        nc.vector.tensor_add(
                            out=v_sb[:, kc, :, 0:HD],
                            in0=ps[:, 0:DL].rearrange("p (h d) -> p h d", h=HL),
                            in1=bv_rep[:],
                        )

                return [(428, lambda hi=hi: h_unit(hi)) for hi in range(2)]

            def pv_norm_unit(h, pan, qc, ex, o_qd):
                """PV for one 128-q chunk + normalize into o_qd (bf16)."""
                ps = cpool.tile([P, 512], F32, tag="c")
                pso = ps[:, 0 : HD + 1]
                for kc in range(NKC):
                    nc.tensor.matmul(
                        pso,
                        lhsT=ex[:, kc, qc * P : (qc + 1) * P],
                        rhs=v_sb[:, kc, h, :],
                        start=(kc == 0),
                        stop=(kc == NKC - 1),
                    )
                rcp = rcpool.tile([P, 1], F32, tag="rcp")
                nc.vector.reciprocal(out=rcp[:], in_=pso[:, HD : HD + 1])
                nc.vector.tensor_scalar(
                    out=o_qd[:, qc, h * HD : (h + 1) * HD],
                    in0=pso[:, 0:HD],
                    scalar1=rcp[:],
                    scalar2=None,
                    op0=mybir.AluOpType.mult,
                )

            def tc_units(pan, qc, o_qd, oT_pan, tail):
                """Transpose + C for one q-chunk, as 3 filler units.
                In the tail, one C eviction moves to the otherwise-idle Act."""
                dve_evict = lambda out, in_: nc.vector.tensor_copy(out=out, in_=in_)

                def t_unit():
                    for fc in range(2):
                        tp = tpool.tile([P, P], BF16, tag="t")
                        nc.tensor.transpose(
                            tp[:], o_qd[:, qc, fc * P : (fc + 1) * P], ident[:]
                        )
                        dve_evict(out=oT_pan[:, fc, qc * P : (qc + 1) * P], in_=tp[:])

                cell = [None]

                def c_unit(half):
                    if half == 0:
                        cell[0] = csb.tile([P, 2, 512], BF16, tag="co", name=nm("co"))
                    cp = cpool.tile([P, 512], F32, tag="c")
                    for fc in range(2):
                        nc.tensor.matmul(
                            cp[:],
                            lhsT=oT_pan[:, fc, qc * P : (qc + 1) * P],
                            rhs=wo_sb[:, fc, half * 512 : (half + 1) * 512],
                            start=(fc == 0),
                            stop=(fc == 1),
                        )
                    if tail and half == 1:
                        nc.scalar.copy(out=cell[0][:, half, :], in_=cp[:])
                    else:
                        dve_evict(out=cell[0][:, half, :], in_=cp[:])
                    if half == 1:
                        q0 = pan * PAN + qc * P
                        nc.sync.dma_start(out=OUT[q0 : q0 + P, :], in_=cell[0][:])

                return [(120, t_unit), (426, lambda: c_unit(0)), (426, lambda: c_unit(1))]

            # ---- startup projections: K(dc0) tokens 0..511, Q(p0,dc0) ----
            for est, f in proj_quarters(
                kT[:, 0, 0:512], slice(0, P), wk_sb,
                bk_sb[:, 0:1], [ktA], slice(0, 512),
            ):
                f()
            for u in range(2):
                for est, f in proj_quarters(
                    qT[:, 0, u * 512 : (u + 1) * 512], slice(0, P), wq_sb,
                    bq_sb[:, 0:1], [qt0], slice(u * 512, (u + 1) * 512),
                ):
                    f()

            # K(dc0) for tokens 1024..2047 trails into the B loop as fillers
            fillers = []
            est_state = {"pe": 18000.0, "act": 0.0, "hist": []}

            def addf(units, key=None):
                for est, f in units:
                    fillers.append((est, key, f))

            addf(
                proj_quarters(
                    kT[:, 0, 512:1024], slice(0, P),
                    wk_sb, bk_sb[:, 0:1], [ktA], slice(512, 1024),
                ),
                key="kdc0a",
            )
            addf(
                proj_quarters(
                    kT[:, 0, PAN : PAN + 512], slice(0, P),
                    wk_sb, bk_sb[:, 0:1], [ktB], slice(0, 512),
                ),
                key="kdc0b1",
            )
            # Q(p0, dc1)
            for u in range(2):
                addf(
                    proj_quarters(
                        qT[:, 1, u * 512 : (u + 1) * 512], slice(P, 2 * P), wq_sb,
                        bq_sb[:, 1:2], [qt0], slice(u * 512, (u + 1) * 512),
                    ),
                    key="qp0",
                )
            # V + K(dc1) straight from the resident ktA/ktB panels
            for t4 in range(8):
                addf(vproj_halves([ktA], t4, t4), key="vk")
                if t4 == 1:
                    # second half of K(dc0) (needs the later ktB half-DMA)
                    addf(
                        proj_quarters(
                            kT[:, 0, PAN + 512 : N], slice(0, P),
                            wk_sb, bk_sb[:, 0:1], [ktB], slice(512, 1024),
                        ),
                        key="kdc0b2",
                    )
                if t4 == 3:
                    addf(
                        proj_quarters(
                            kT[:, 1, 0:512], slice(P, 2 * P), wk_sb,
                            bk_sb[:, 1:2], [ktA], slice(0, 512),
                        ),
                        key="vk",
                    )
            addf(
                proj_quarters(
                    kT[:, 1, 512:1024], slice(P, 2 * P), wk_sb, bk_sb[:, 1:2],
                    [ktA], slice(512, 1024),
                ),
                key="vk",
            )
            for t4 in range(8):
                addf(vproj_halves([ktB], t4, 8 + t4), key="vkb")
                if t4 == 3:
                    addf(
                        proj_quarters(
                            kT[:, 1, PAN : PAN + 512], slice(P, 2 * P), wk_sb,
                            bk_sb[:, 1:2], [ktB], slice(0, 512),
                        ),
                        key="vkb",
                    )
            addf(
                proj_quarters(
                    kT[:, 1, PAN + 512 : N], slice(P, 2 * P), wk_sb, bk_sb[:, 1:2],
                    [ktB], slice(512, 1024),
                ),
                key="vkb",
            )
            # qt panel 1 (4th 'pan' ring alloc frees ktA) + Q(p1) both dc
            qt1 = [None]

            def alloc_dma_qt1():
                qt1[0] = t = panpool.tile([P, KO, PAN], BF16, tag="pan", name="qt1")
                for s in range(2):
                    nc.sync.dma_start(
                        out=t[:, :, s * 512 : (s + 1) * 512],
                        in_=qt_r[:, :, PAN + s * 512 : PAN + (s + 1) * 512],
                    )

            addf([(0, alloc_dma_qt1)], key="qp1")
            for dc in range(2):
                for u in range(2):
                    addf(
                        proj_quarters(
                            qT[:, dc, PAN + u * 512 : PAN + (u + 1) * 512],
                            slice(dc * P, (dc + 1) * P), wq_sb,
                            bq_sb[:, dc : dc + 1], qt1, slice(u * 512, (u + 1) * 512),
                        ),
                        key="qp1",
                    )
            addf([(0, late_dmas)])

            def emit_fillers():
                """Pop fillers while the estimated PE clock trails the
                estimated Act clock (keeps exp gapless without bursting)."""
                while fillers:
                    est, key, f = fillers[0]
                    if est_state["pe"] + est > est_state["act"] - 900:
                        break
                    fillers.pop(0)
                    f()
                    est_state["pe"] += est

            def drain_through(keys):
                """Force-emit queue entries up to the last entry whose key is
                in `keys` (emission-order deadlines at head boundaries)."""
                if not any(k in keys for _, k, _ in fillers):
                    return
                last = max(i for i, (_, k, _) in enumerate(fillers) if k in keys)
                for est, _, f in fillers[: last + 1]:
                    f()
                    est_state["pe"] += est
                del fillers[: last + 1]

            # ---- fused B/C pipeline ----
            pending = []
            for pan in range(NPAN):
                o_qd = oqpool.tile([P, QCP, DL], BF16, tag="oq", name=f"oq{pan}")
                oT_pan = otpool.tile([P, 2, PAN], BF16, tag="ot", name=f"ot{pan}")
                for h in range(HL):
                    g = pan * HL + h
                    dc, po = h // 2, (h % 2) * HD
                    # emission-order deadlines (before this head's first E):
                    # kT(dc1) writers before h2; qT(p1) before panel 1.
                    need = set()
                    if pan == 0 and h == 2:
                        need.add("vk")
                    if pan == 1 and h == 0:
                        need.add("qp1")
                    drain_through(need)
                    deferred_vkb = pan == 0 and h == 2
                    ex = expool.tile([P, NKC, PAN], BF16, tag="ex", name=f"ex{pan}_{h}")
                    for kc in range(NKC):
                        ps = pepool.tile([P, PAN], F32, tag="e")
                        for eh in range(2):
                            nc.tensor.matmul(
                                ps[:, eh * 512 : (eh + 1) * 512],
                                lhsT=kT[po : po + HD, dc, kc * P : (kc + 1) * P],
                                rhs=qT[
                                    po : po + HD,
                                    dc,
                                    pan * PAN + eh * 512 : pan * PAN + (eh + 1) * 512,
                                ],
                                start=True,
                                stop=True,
                            )
                        hist = est_state["hist"]
                        if len(hist) >= 2:
                            est_state["pe"] = max(est_state["pe"], hist[-2] + 250)
                        est_state["pe"] += 427
                        nc.scalar.activation(
                            out=ex[:, kc, :], in_=ps[:], func=AF.Exp, scale=SCALE
                        )
                        prev = hist[-1] if hist else est_state["pe"] + 870
                        hist.append(max(prev, est_state["pe"] + 250) + 1038)
                        est_state["act"] = hist[-1]
                        if kc == 3 and pending:
                            vk_idx = [
                                i
                                for i, (_, k, _) in enumerate(fillers)
                                if k in ("vk", "vkb")
                            ]
                            at = (vk_idx[-1] + 1) if vk_idx else 0
                            fillers[at:at] = pending
                            pending.clear()
                        if pan == 0 and h == 0 and kc == 2:
                            drain_through({"kdc0a"})
                        if pan == 0 and h == 0 and kc == 6:
                            drain_through({"kdc0b1", "kdc0b2"})
                        if pan == 0 and h == 1 and kc == 0:
                            drain_through({"kdc0b1", "kdc0b2"})
                        if kc == 2 and g >= 2:
                            drain_through({("pv", g - 2)})
                        if pan == 0 and h == 3 and kc == 2:
                            drain_through({"vk23"})
                        if deferred_vkb and kc == 5:
                            drain_through({"vkb"})
                        emit_fillers()
                        last_head = pan == NPAN - 1 and h == HL - 1
                        if last_head and kc >= 14:
                            mt = m_half[(pan, 1)]
                            k0 = kc - 8
                            nc.vector.tensor_mul(
                                out=ex[:, kc : kc + 1, :],
                                in0=ex[:, kc : kc + 1, :],
                                in1=mt[:, k0 : k0 + 1, :],
                            )
                        elif kc % 2 == 1:
                            pair = (kc - 1) // 2
                            mt = m_half[(pan, kc // 8)]
                            k0 = kc - 1 - (kc // 8) * 8
                            nc.vector.tensor_mul(
                                out=ex[:, kc - 1 : kc + 1, :],
                                in0=ex[:, kc - 1 : kc + 1, :],
                                in1=mt[:, k0 : k0 + 2, :],
                            )
                        if pan == 0 and h == 3 and kc == 2:
                            mask_dma(1, 0)
                        if pan == 0 and h == 3 and kc == 10:
                            mask_dma(1, 1)
                    # queue this head's PV into `pending`: released at kc==3
                    # of the next head, after that head's last masks have had
                    # time to finish (PV's kc14/15 matmuls depend on them)
                    for qc in range(QCP):
                        pending.append(
                            (
                                432,
                                ("pv", g),
                                lambda h=h, pan=pan, qc=qc, ex=ex, o_qd=o_qd: pv_norm_unit(
                                    h, pan, qc, ex, o_qd
                                ),
                            )
                        )
                    if h == HL - 1:
                        for qc in range(QCP):
                            for est, f in tc_units(
                                pan, qc, o_qd, oT_pan, tail=(pan == NPAN - 1)
                            ):
                                pending.append((est, "tc", f))

            # drain remaining fillers; software-pipeline the last panel's
            # PV -> T -> C chains (PV j+1 before T j, C j-1 after T j)
            while fillers:
                fillers.pop(0)[2]()
            pvs = [f for _, k, f in pending if k != "tc"]
            tcs = [f for _, k, f in pending if k == "tc"]
            tcg = [tcs[i * 3 : (i + 1) * 3] for i in range(len(tcs) // 3)]
            pending.clear()
            stages = []
            for j in range(len(pvs)):
                stages.append(pvs[j])
                if j >= 1 and j - 1 < len(tcg):
                    stages.extend(tcg[j - 1][:1])  # transpose of j-1
                if j >= 2 and j - 2 < len(tcg):
                    stages.extend(tcg[j - 2][1:])  # C halves of j-2
            for j in range(max(0, len(pvs) - 1), len(tcg)):
                stages.extend(tcg[j][:1])
            for j in range(max(0, len(pvs) - 2), len(tcg)):
                stages.extend(tcg[j][1:])
            for f in stages:
                f()

    nc.finalize()
    return nc


_NC = None


def _get_nc():
    global _NC
    if _NC is None:
        _NC = build_nc()
    return _NC


def make_in_maps(Q, K, mask, Wq, bq, Wk, bk, Wv, bv, Wo, bo):
    Q = np.asarray(Q, np.float32)
    K = np.asarray(K, np.float32)
    mask = np.asarray(mask)
    Wq = np.asarray(Wq, np.float32)
    Wk = np.asarray(Wk, np.float32)
    Wv = np.asarray(Wv, np.float32)
    Wo = np.asarray(Wo, np.float32)
    qt = [np.ascontiguousarray(Q[b].T).astype(ml_dtypes.bfloat16) for b in range(B)]
    kt = [np.ascontiguousarray(K[b].T).astype(ml_dtypes.bfloat16) for b in range(B)]
    # [pan, p, kc, q'] = mask[b].T[kc*128+p, pan*1024+q']
    mt = [
        np.ascontiguousarray(
            mask[b].T.reshape(NKC, P, NPAN, PAN).transpose(2, 1, 0, 3)
        ).astype(ml_dtypes.bfloat16)
        for b in range(B)
    ]
    ident = np.eye(P, dtype=np.float32).astype(ml_dtypes.bfloat16)
    in_maps = []
    for c in range(8):
        b, hg = divmod(c, 4)
        cols = slice(hg * DL, (hg + 1) * DL)
        in_maps.append(
            {
                "qt": qt[b],
                "kt": kt[b],
                "mt": mt[b],
                "wqt": np.ascontiguousarray(Wq[cols, :].T).astype(ml_dtypes.bfloat16),
                "wkt": np.ascontiguousarray(Wk[cols, :].T).astype(ml_dtypes.bfloat16),
                "wvt": np.ascontiguousarray(Wv[cols, :].T).astype(ml_dtypes.bfloat16),
                "wot": np.ascontiguousarray(Wo[:, cols].T).astype(ml_dtypes.bfloat16),
                "bq": np.ascontiguousarray(np.asarray(bq, np.float32)[cols]),
                "bk": np.ascontiguousarray(np.asarray(bk, np.float32)[cols]),
                "bv": np.ascontiguousarray(np.asarray(bv, np.float32)[cols]),
                "ident": ident,
            }
        )
    return in_maps


def assemble(results, bo):
    O = np.zeros((B, N, D), np.float32)
    for c in range(8):
        b = c // 4
        O[b] += np.asarray(results[c]["out"], np.float32)
    O += np.asarray(bo, np.float32)[None, None, :]
    return O


def kernel(Q, K, mask, Wq, bq, Wk, bk, Wv, bv, Wo, bo):
    nc = _get_nc()
    in_maps = make_in_maps(Q, K, mask, Wq, bq, Wk, bk, Wv, bv, Wo, bo)
    res = run_bass_kernel_spmd(nc, in_maps, core_ids=list(range(8)))
    return assemble(res.results, bo)


# revision 46
# speedup vs baseline: 1.2661x; 1.0180x over previous
"""Trainium2 Bass kernel for nn_MHA_36584531427723.

Sharding: 8 cores = 2 batches x 4 head-groups (4 heads of 64 dims per core).
Each core: Q/K/V projections for its 256 features, attention for its 4 heads,
partial output projection (its 256 rows of Wo^T). Host sums 4 partials + bo.

Single fused pipeline per core (vs the phase-sequential baseline):
  - Scores computed transposed E^T[k,q] (contraction=head_dim on partitions).
  - exp on the Act engine in 1024-wide tiles; it is the near-critical path, so
    the PE stream is emitted as E-matmuls interleaved with small "filler"
    units (projections, PV, transpose, C) kept under ~450ns each so the exp
    stream never starves; a budget scheduler pops fillers between E-matmuls.
  - PV in [q, 65] orientation (out free = 65 -> half the PE cost of the
    baseline's [65, q] form); a ones column in V gives softmax denominators.
  - o normalized per-q via DVE tensor_scalar with per-partition reciprocal,
    PE-transposed (bf16) to feature-major for the C matmul.
  - masked softmax: exp then tensor_tensor multiply with bf16 0/1 mask
    (DVE 2x_1p; 1/4 of tiles on GpSimd); max-subtraction and +eps dropped
    (|E|~<2; relative effect ~1e-11, same argument as the baseline).
  - OUT written bf16 (host accumulates partials in f32).
"""

import numpy as np
import ml_dtypes

import concourse.bacc as bacc
import concourse.bass as bass  # noqa: F401
import concourse.mybir as mybir
import concourse.tile as tile
from concourse.bass_utils import run_bass_kernel_spmd

B, N, D = 2, 2048, 1024
H = 16
HD = 64
HL = 4  # heads per core
DL = HL * HD  # 256 local features
P = 128
KO = D // P  # 8 contraction chunks for projections
NKC = N // P  # 16 k-token chunks
PAN = 1024  # q panel width in phase B
NPAN = N // PAN  # 2
QCP = PAN // P  # 8 q-chunks per panel
SCALE = 1.0 / 32.0  # 1/sqrt(DIM_V)

F32 = mybir.dt.float32
BF16 = mybir.dt.bfloat16
AF = mybir.ActivationFunctionType


def build_nc():
    nc = bacc.Bacc(None, target_bir_lowering=False)
    QT = nc.dram_tensor("qt", (D, N), BF16, kind="ExternalInput")
    KT = nc.dram_tensor("kt", (D, N), BF16, kind="ExternalInput")
    # mask, transposed + panel-major: [pan, p, kc, q'] = mask[b].T[kc*128+p, pan*1024+q']
    MT = nc.dram_tensor("mt", (NPAN, P, NKC, PAN), BF16, kind="ExternalInput")
    WQT = nc.dram_tensor("wqt", (D, DL), BF16, kind="ExternalInput")
    WKT = nc.dram_tensor("wkt", (D, DL), BF16, kind="ExternalInput")
    WVT = nc.dram_tensor("wvt", (D, DL), BF16, kind="ExternalInput")
    WOT = nc.dram_tensor("wot", (DL, D), BF16, kind="ExternalInput")
    BQ = nc.dram_tensor("bq", (DL,), F32, kind="ExternalInput")
    BK = nc.dram_tensor("bk", (DL,), F32, kind="ExternalInput")
    BV = nc.dram_tensor("bv", (DL,), F32, kind="ExternalInput")
    IDENT = nc.dram_tensor("ident", (P, P), BF16, kind="ExternalInput")
    OUT = nc.dram_tensor("out", (N, D), BF16, kind="ExternalOutput")

    qt_r = QT[:].rearrange("(ko p) q -> p ko q", p=P)
    kt_r = KT[:].rearrange("(ko p) q -> p ko q", p=P)

    uid = [0]

    def nm(pfx):
        uid[0] += 1
        return f"{pfx}{uid[0]}"

    with tile.TileContext(nc) as tc:
        with (
            tc.tile_pool(name="persist", bufs=1) as persist,
            tc.tile_pool(name="panpool", bufs=3) as panpool,
            tc.tile_pool(name="mpool", bufs=2) as mpool,
            tc.tile_pool(name="expool", bufs=4) as expool,
            tc.tile_pool(name="oqpool", bufs=2) as oqpool,
            tc.tile_pool(name="otpool", bufs=1) as otpool,
            tc.tile_pool(name="csb", bufs=2) as csb,
            tc.tile_pool(name="rcpool", bufs=2) as rcpool,
            tc.tile_pool(name="pepool", bufs=2, space="PSUM") as pepool,
            tc.tile_pool(name="cpool", bufs=2, space="PSUM") as cpool,
            tc.tile_pool(name="tpool", bufs=2, space="PSUM") as tpool,
        ):
            # ---- persistent SBUF ----
            qT = persist.tile([P, 2, N], BF16, tag="qT")
            kT = persist.tile([P, 2, N], BF16, tag="kT")
            v_sb = persist.tile([P, NKC, HL, HD + 1], BF16, tag="v")
            wq_sb = persist.tile([P, KO, DL], BF16, tag="wq")
            wk_sb = persist.tile([P, KO, DL], BF16, tag="wk")
            wv_sb = persist.tile([P, KO, DL], BF16, tag="wv")
            wo_sb = persist.tile([P, 2, D], BF16, tag="wo")
            bq_sb = persist.tile([P, 2], F32, tag="bq")
            bk_sb = persist.tile([P, 2], F32, tag="bk")
            bv_rep = persist.tile([P, HL, HD], F32, tag="bv")
            ident = persist.tile([P, P], BF16, tag="ident")

            m_half = {}

            def mask_dma(pan, half):
                """Mask half-panel [128, 8 kc, 1024]; two quad-DMAs."""
                m_half[(pan, half)] = mt = mpool.tile(
                    [P, NKC // 2, PAN], BF16, tag="m", name=f"m{pan}_{half}"
                )
                for qd in range(2):
                    nc.sync.dma_start(
                        out=mt[:, qd * 4 : (qd + 1) * 4, :],
                        in_=MT[pan, :, half * 8 + qd * 4 : half * 8 + (qd + 1) * 4, :],
                    )

            # ---- startup DMAs, ordered for earliest gapless exp stream ----
            def half_dma(t, src_r, col0, s):
                nc.sync.dma_start(
                    out=t[:, :, s * 512 : (s + 1) * 512],
                    in_=src_r[:, :, col0 + s * 512 : col0 + (s + 1) * 512],
                )

            nc.sync.dma_start(
                out=wk_sb[:], in_=WKT[:].rearrange("(ko p) m -> p ko m", p=P)
            )
            ktA = panpool.tile([P, KO, PAN], BF16, tag="pan", name="ktA")
            half_dma(ktA, kt_r, 0, 0)
            nc.sync.dma_start(
                out=wq_sb[:], in_=WQT[:].rearrange("(ko p) m -> p ko m", p=P)
            )
            qt0 = panpool.tile([P, KO, PAN], BF16, tag="pan", name="qt0")
            half_dma(qt0, qt_r, 0, 0)
            half_dma(qt0, qt_r, 0, 1)
            nc.sync.dma_start(out=bk_sb[:], in_=BK[:].rearrange("(c p) -> p c", p=P))
            nc.sync.dma_start(out=bq_sb[:], in_=BQ[:].rearrange("(c p) -> p c", p=P))
            half_dma(ktA, kt_r, 0, 1)
            nc.sync.dma_start(
                out=wv_sb[:], in_=WVT[:].rearrange("(ko p) m -> p ko m", p=P)
            )
            nc.sync.dma_start(
                out=bv_rep[:],
                in_=BV[:].rearrange("(h d) -> h d", h=HL)[None].to_broadcast(
                    (P, HL, HD)
                ),
            )
            ktB = panpool.tile([P, KO, PAN], BF16, tag="pan", name="ktB")
            half_dma(ktB, kt_r, PAN, 0)
            half_dma(ktB, kt_r, PAN, 1)
            mask_dma(0, 0)  # 2 quad DMAs
            mask_dma(0, 1)
            nc.vector.memset(v_sb[:, :, :, HD : HD + 1], 1.0)

            def late_dmas():
                nc.sync.dma_start(
                    out=wo_sb[:], in_=WOT[:].rearrange("(cc p) n -> p cc n", p=P)
                )
                nc.sync.dma_start(out=ident[:], in_=IDENT[:])

            # ---- helper emitters ----
            def proj_quarters(dst, fslice, w_sb, bias_ap, pan_cell, panslice):
                """One [128 x 512] projection as a list of 4 filler units
                (2 ko-steps each, ~426ns PE) sharing one cpool tile.
                pan_cell: 1-elem list deref'd lazily (reload tiles)."""
                cell = [None]

                def q_unit(qi):
                    if qi == 0:
                        cell[0] = cpool.tile([P, 512], F32, tag="c", name=nm("pq"))
                    ps = cell[0]
                    for ko in range(2 * qi, 2 * qi + 2):
                        nc.tensor.matmul(
                            ps[:],
                            lhsT=w_sb[:, ko, fslice],
                            rhs=pan_cell[0][:, ko, panslice],
                            start=(ko == 0),
                            stop=(ko == KO - 1),
                        )
                    if qi == 3:
                        nc.vector.tensor_scalar_add(dst, ps[:], bias_ap)

                return [(426, lambda qi=qi: q_unit(qi)) for qi in range(4)]

            def vproj_halves(pan_cell, t4, kc):
                """V for one 128-token chunk (all 4 heads) as 2 filler units."""
                cell = [None]

                def h_unit(hi):
                    if hi == 0:
                        cell[0] = cpool.tile([P, 512], F32, tag="c", name=nm("pq"))
                    ps = cell[0]
                    for ko in range(4 * hi, 4 * hi + 4):
                        nc.tensor.matmul(
                            ps[:, 0:DL],
                            lhsT=pan_cell[0][:, ko, t4 * P : (t4 + 1) * P],
                            rhs=wv_sb[:, ko, :],
                            start=(ko == 0),
                            stop=(ko == KO - 1),
                        )
                    if hi == 1:
                        nc.vector.tensor_add(
                            out=v_sb[:, kc, :, 0:HD],
                            in0=ps[:, 0:DL].rearrange("p (h d) -> p h d", h=HL),
                            in1=bv_rep[:],
                        )

                return [(428, lambda hi=hi: h_unit(hi)) for hi in range(2)]

            def pv_norm_unit(h, pan, qc, ex_lo, ex_hi, o_qd):
                """PV for one 128-q chunk + normalize into o_qd (bf16)."""
                ps = cpool.tile([P, 512], F32, tag="c")
                pso = ps[:, 0 : HD + 1]
                for kc in range(NKC):
                    exh = ex_lo if kc < 8 else ex_hi
                    nc.tensor.matmul(
                        pso,
                        lhsT=exh[:, kc % 8, qc * P : (qc + 1) * P],
                        rhs=v_sb[:, kc, h, :],
                        start=(kc == 0),
                        stop=(kc == NKC - 1),
                    )
                rcp = rcpool.tile([P, 1], F32, tag="rcp")
                nc.vector.reciprocal(out=rcp[:], in_=pso[:, HD : HD + 1])
                nc.vector.tensor_scalar(
                    out=o_qd[:, qc, h * HD : (h + 1) * HD],
                    in0=pso[:, 0:HD],
                    scalar1=rcp[:],
                    scalar2=None,
                    op0=mybir.AluOpType.mult,
                )

            def tc_units(pan, qc, o_qd, oT_pan, tail):
                """Transpose + C for one q-chunk, as 3 filler units.
                In the tail, one C eviction moves to the otherwise-idle Act."""
                dve_evict = lambda out, in_: nc.vector.tensor_copy(out=out, in_=in_)

                def t_unit():
                    for fc in range(2):
                        tp = tpool.tile([P, P], BF16, tag="t")
                        nc.tensor.transpose(
                            tp[:], o_qd[:, qc, fc * P : (fc + 1) * P], ident[:]
                        )
                        dve_evict(out=oT_pan[:, fc, qc * P : (qc + 1) * P], in_=tp[:])

                cell = [None]

                def c_unit(half):
                    if half == 0:
                        cell[0] = csb.tile([P, 2, 512], BF16, tag="co", name=nm("co"))
                    cp = cpool.tile([P, 512], F32, tag="c")
                    for fc in range(2):
                        nc.tensor.matmul(
                            cp[:],
                            lhsT=oT_pan[:, fc, qc * P : (qc + 1) * P],
                            rhs=wo_sb[:, fc, half * 512 : (half + 1) * 512],
                            start=(fc == 0),
                            stop=(fc == 1),
                        )
                    if tail and half == 1:
                        nc.scalar.copy(out=cell[0][:, half, :], in_=cp[:])
                    else:
                        dve_evict(out=cell[0][:, half, :], in_=cp[:])
                    if half == 1:
                        q0 = pan * PAN + qc * P
                        nc.sync.dma_start(out=OUT[q0 : q0 + P, :], in_=cell[0][:])

                return [(120, t_unit), (426, lambda: c_unit(0)), (426, lambda: c_unit(1))]

            # ---- startup projections: K(dc0) tokens 0..511, Q(p0,dc0) ----
            for est, f in proj_quarters(
                kT[:, 0, 0:512], slice(0, P), wk_sb,
                bk_sb[:, 0:1], [ktA], slice(0, 512),
            ):
                f()
            for u in range(2):
                for est, f in proj_quarters(
                    qT[:, 0, u * 512 : (u + 1) * 512], slice(0, P), wq_sb,
                    bq_sb[:, 0:1], [qt0], slice(u * 512, (u + 1) * 512),
                ):
                    f()

            # K(dc0) for tokens 1024..2047 trails into the B loop as fillers
            fillers = []
            est_state = {"pe": 18000.0, "act": 0.0, "hist": []}

            def addf(units, key=None):
                for est, f in units:
                    fillers.append((est, key, f))

            addf(
                proj_quarters(
                    kT[:, 0, 512:1024], slice(0, P),
                    wk_sb, bk_sb[:, 0:1], [ktA], slice(512, 1024),
                ),
                key="kdc0a",
            )
            addf(
                proj_quarters(
                    kT[:, 0, PAN : PAN + 512], slice(0, P),
                    wk_sb, bk_sb[:, 0:1], [ktB], slice(0, 512),
                ),
                key="kdc0b1",
            )
            # Q(p0, dc1)
            for u in range(2):
                addf(
                    proj_quarters(
                        qT[:, 1, u * 512 : (u + 1) * 512], slice(P, 2 * P), wq_sb,
                        bq_sb[:, 1:2], [qt0], slice(u * 512, (u + 1) * 512),
                    ),
                    key="qp0",
                )
            # V + K(dc1) straight from the resident ktA/ktB panels
            for t4 in range(8):
                addf(vproj_halves([ktA], t4, t4), key="vk")
                if t4 == 1:
                    # second half of K(dc0) (needs the later ktB half-DMA)
                    addf(
                        proj_quarters(
                            kT[:, 0, PAN + 512 : N], slice(0, P),
                            wk_sb, bk_sb[:, 0:1], [ktB], slice(512, 1024),
                        ),
                        key="kdc0b2",
                    )
                if t4 == 3:
                    addf(
                        proj_quarters(
                            kT[:, 1, 0:512], slice(P, 2 * P), wk_sb,
                            bk_sb[:, 1:2], [ktA], slice(0, 512),
                        ),
                        key="vk",
                    )
            addf(
                proj_quarters(
                    kT[:, 1, 512:1024], slice(P, 2 * P), wk_sb, bk_sb[:, 1:2],
                    [ktA], slice(512, 1024),
                ),
                key="vk",
            )
            for t4 in range(8):
                addf(vproj_halves([ktB], t4, 8 + t4), key="vkb")
                if t4 == 3:
                    addf(
                        proj_quarters(
                            kT[:, 1, PAN : PAN + 512], slice(P, 2 * P), wk_sb,
                            bk_sb[:, 1:2], [ktB], slice(0, 512),
                        ),
                        key="vkb",
                    )
            addf(
                proj_quarters(
                    kT[:, 1, PAN + 512 : N], slice(P, 2 * P), wk_sb, bk_sb[:, 1:2],
                    [ktB], slice(512, 1024),
                ),
                key="vkb",
            )
            # qt panel 1 (4th 'pan' ring alloc frees ktA) + Q(p1) both dc
            qt1 = [None]

            def alloc_dma_qt1():
                qt1[0] = t = panpool.tile([P, KO, PAN], BF16, tag="pan", name="qt1")
                for s in range(2):
                    nc.sync.dma_start(
                        out=t[:, :, s * 512 : (s + 1) * 512],
                        in_=qt_r[:, :, PAN + s * 512 : PAN + (s + 1) * 512],
                    )

            addf([(0, alloc_dma_qt1)], key="qp1")
            for dc in range(2):
                for u in range(2):
                    addf(
                        proj_quarters(
                            qT[:, dc, PAN + u * 512 : PAN + (u + 1) * 512],
                            slice(dc * P, (dc + 1) * P), wq_sb,
                            bq_sb[:, dc : dc + 1], qt1, slice(u * 512, (u + 1) * 512),
                        ),
                        key="qp1",
                    )
            addf([(0, late_dmas)])

            def emit_fillers():
                """Pop fillers while the estimated PE clock trails the
                estimated Act clock (keeps exp gapless without bursting)."""
                while fillers:
                    est, key, f = fillers[0]
                    if est_state["pe"] + est > est_state["act"] - 900:
                        break
                    fillers.pop(0)
                    f()
                    est_state["pe"] += est

            def drain_through(keys):
                """Force-emit queue entries up to the last entry whose key is
                in `keys` (emission-order deadlines at head boundaries)."""
                if not any(k in keys for _, k, _ in fillers):
                    return
                last = max(i for i, (_, k, _) in enumerate(fillers) if k in keys)
                for est, _, f in fillers[: last + 1]:
                    f()
                    est_state["pe"] += est
                del fillers[: last + 1]

            # ---- fused B/C pipeline ----
            # ex half-tile slot sources per global head: 'x'=expool(4-ring),
            # 'p'=panpool 'pan' slot (same 16KB shape; kt/qt panels dead by
            # then). This 7-slot rotation removes the PV(h-2)-before-exp(h)
            # deadline of the old 2-deep full-tile ring.
            EX_SRC = [
                ("x", "x"), ("x", "x"), ("p", "p"), ("x", "x"),
                ("x", "x"), ("p", "p"), ("p", "x"), ("x", "x"),
            ]
            pending = []
            for pan in range(NPAN):
                o_qd = oqpool.tile([P, QCP, DL], BF16, tag="oq", name=f"oq{pan}")
                oT_pan = otpool.tile([P, 2, PAN], BF16, tag="ot", name=f"ot{pan}")
                for h in range(HL):
                    g = pan * HL + h
                    dc, po = h // 2, (h % 2) * HD
                    # emission-order deadlines (before this head's first E):
                    # kT(dc1) writers before h2; qT(p1) before panel 1.
                    need = set()
                    if pan == 0 and h == 2:
                        need.add("vk")
                    if pan == 0 and h == 3:
                        need.add("vkb")
                    if pan == 1 and h == 0:
                        need.add("qp1")
                    drain_through(need)
                    deferred_vkb = pan == 0 and h == 2
                    src_lo, src_hi = EX_SRC[g]
                    def _ex_alloc(srcc, half):
                        if srcc == "x":
                            return expool.tile(
                                [P, NKC // 2, PAN], BF16, tag="exh",
                                name=f"ex{pan}_{h}_{half}",
                            )
                        return panpool.tile(
                            [P, KO, PAN], BF16, tag="pan", name=f"exp{pan}_{h}_{half}"
                        )
                    ex_lo = _ex_alloc(src_lo, 0)
                    ex_hi = _ex_alloc(src_hi, 1)
                    for kc in range(NKC):
                        ps = pepool.tile([P, PAN], F32, tag="e")
                        for eh in range(2):
                            nc.tensor.matmul(
                                ps[:, eh * 512 : (eh + 1) * 512],
                                lhsT=kT[po : po + HD, dc, kc * P : (kc + 1) * P],
                                rhs=qT[
                                    po : po + HD,
                                    dc,
                                    pan * PAN + eh * 512 : pan * PAN + (eh + 1) * 512,
                                ],
                                start=True,
                                stop=True,
                            )
                        hist = est_state["hist"]
                        if len(hist) >= 2:
                            est_state["pe"] = max(est_state["pe"], hist[-2] + 250)
                        est_state["pe"] += 427
                        exh = ex_lo if kc < 8 else ex_hi
                        nc.scalar.activation(
                            out=exh[:, kc % 8, :], in_=ps[:], func=AF.Exp, scale=SCALE
                        )
                        prev = hist[-1] if hist else est_state["pe"] + 870
                        hist.append(max(prev, est_state["pe"] + 250) + 1038)
                        est_state["act"] = hist[-1]
                        if kc == 3 and pending:
                            vk_idx = [
                                i
                                for i, (_, k, _) in enumerate(fillers)
                                if k in ("vk", "vkb")
                            ]
                            at = (vk_idx[-1] + 1) if vk_idx else 0
                            fillers[at:at] = pending
                            pending.clear()
                        if pan == 0 and h == 0 and kc == 2:
                            drain_through({"kdc0a"})
                        if pan == 0 and h == 0 and kc == 6:
                            drain_through({"kdc0b1", "kdc0b2"})
                        if pan == 0 and h == 1 and kc == 0:
                            drain_through({"kdc0b1", "kdc0b2"})
                        if kc == 2 and g >= 2:
                            drain_through({("pv", g - 2)})

                        if deferred_vkb and kc == 5:
                            drain_through({"vkb"})
                        emit_fillers()
                        last_head = pan == NPAN - 1 and h == HL - 1
                        exh = ex_lo if kc < 8 else ex_hi
                        if last_head and kc >= 14:
                            mt = m_half[(pan, 1)]
                            k0 = kc - 8
                            nc.vector.tensor_mul(
                                out=exh[:, k0 : k0 + 1, :],
                                in0=exh[:, k0 : k0 + 1, :],
                                in1=mt[:, k0 : k0 + 1, :],
                            )
                        elif kc % 2 == 1:
                            mt = m_half[(pan, kc // 8)]
                            k0 = kc - 1 - (kc // 8) * 8
                            nc.vector.tensor_mul(
                                out=exh[:, k0 : k0 + 2, :],
                                in0=exh[:, k0 : k0 + 2, :],
                                in1=mt[:, k0 : k0 + 2, :],
                            )
                        if pan == 0 and h == 3 and kc == 2:
                            mask_dma(1, 0)
                        if pan == 0 and h == 3 and kc == 10:
                            mask_dma(1, 1)
                    # queue this head's PV into `pending`: released at kc==3
                    # of the next head, after that head's last masks have had
                    # time to finish (PV's kc14/15 matmuls depend on them)
                    for qc in range(QCP):
                        pending.append(
                            (
                                432,
                                ("pv", g),
                                lambda h=h, pan=pan, qc=qc, el=ex_lo, eh2=ex_hi, o_qd=o_qd: pv_norm_unit(
                                    h, pan, qc, el, eh2, o_qd
                                ),
                            )
                        )
                    if h == HL - 1:
                        for qc in range(QCP):
                            for est, f in tc_units(
                                pan, qc, o_qd, oT_pan, tail=(pan == NPAN - 1)
                            ):
                                pending.append((est, "tc", f))

            # drain remaining fillers; software-pipeline the last panel's
            # PV -> T -> C chains (PV j+1 before T j, C j-1 after T j)
            while fillers:
                fillers.pop(0)[2]()
            pvs = [f for _, k, f in pending if k != "tc"]
            tcs = [f for _, k, f in pending if k == "tc"]
            tcg = [tcs[i * 3 : (i + 1) * 3] for i in range(len(tcs) // 3)]
            pending.clear()
            stages = []
            for j in range(len(pvs)):
                stages.append(pvs[j])
                if j >= 1 and j - 1 < len(tcg):
                    stages.extend(tcg[j - 1][:1])  # transpose of j-1
                if j >= 2 and j - 2 < len(tcg):
                    stages.extend(tcg[j - 2][1:])  # C halves of j-2
            for j in range(max(0, len(pvs) - 1), len(tcg)):
                stages.extend(tcg[j][:1])
            for j in range(max(0, len(pvs) - 2), len(tcg)):
                stages.extend(tcg[j][1:])
            for f in stages:
                f()

    nc.finalize()
    return nc


_NC = None


def _get_nc():
    global _NC
    if _NC is None:
        _NC = build_nc()
    return _NC


def make_in_maps(Q, K, mask, Wq, bq, Wk, bk, Wv, bv, Wo, bo):
    Q = np.asarray(Q, np.float32)
    K = np.asarray(K, np.float32)
    mask = np.asarray(mask)
    Wq = np.asarray(Wq, np.float32)
    Wk = np.asarray(Wk, np.float32)
    Wv = np.asarray(Wv, np.float32)
    Wo = np.asarray(Wo, np.float32)
    qt = [np.ascontiguousarray(Q[b].T).astype(ml_dtypes.bfloat16) for b in range(B)]
    kt = [np.ascontiguousarray(K[b].T).astype(ml_dtypes.bfloat16) for b in range(B)]
    # [pan, p, kc, q'] = mask[b].T[kc*128+p, pan*1024+q']
    mt = [
        np.ascontiguousarray(
            mask[b].T.reshape(NKC, P, NPAN, PAN).transpose(2, 1, 0, 3)
        ).astype(ml_dtypes.bfloat16)
        for b in range(B)
    ]
    ident = np.eye(P, dtype=np.float32).astype(ml_dtypes.bfloat16)
    in_maps = []
    for c in range(8):
        b, hg = divmod(c, 4)
        cols = slice(hg * DL, (hg + 1) * DL)
        in_maps.append(
            {
                "qt": qt[b],
                "kt": kt[b],
                "mt": mt[b],
                "wqt": np.ascontiguousarray(Wq[cols, :].T).astype(ml_dtypes.bfloat16),
                "wkt": np.ascontiguousarray(Wk[cols, :].T).astype(ml_dtypes.bfloat16),
                "wvt": np.ascontiguousarray(Wv[cols, :].T).astype(ml_dtypes.bfloat16),
                "wot": np.ascontiguousarray(Wo[:, cols].T).astype(ml_dtypes.bfloat16),
                "bq": np.ascontiguousarray(np.asarray(bq, np.float32)[cols]),
                "bk": np.ascontiguousarray(np.asarray(bk, np.float32)[cols]),
                "bv": np.ascontiguousarray(np.asarray(bv, np.float32)[cols]),
                "ident": ident,
            }
        )
    return in_maps


def assemble(results, bo):
    O = np.zeros((B, N, D), np.float32)
    for c in range(8):
        b = c // 4
        O[b] += np.asarray(results[c]["out"], np.float32)
    O += np.asarray(bo, np.float32)[None, None, :]
    return O


def kernel(Q, K, mask, Wq, bq, Wk, bk, Wv, bv, Wo, bo):
    nc = _get_nc()
    in_maps = make_in_maps(Q, K, mask, Wq, bq, Wk, bk, Wv, bv, Wo, bo)
    res = run_bass_kernel_spmd(nc, in_maps, core_ids=list(range(8)))
    return assemble(res.results, bo)


# revision 62
# speedup vs baseline: 1.2752x; 1.0072x over previous
"""Trainium2 Bass kernel for nn_MHA_36584531427723.

Sharding: 8 cores = 2 batches x 4 head-groups (4 heads of 64 dims per core).
Each core: Q/K/V projections for its 256 features, attention for its 4 heads,
partial output projection (its 256 rows of Wo^T). Host sums 4 partials + bo.

Single fused pipeline per core (vs the phase-sequential baseline):
  - Scores computed transposed E^T[k,q] (contraction=head_dim on partitions).
  - exp on the Act engine in 1024-wide tiles; it is the near-critical path, so
    the PE stream is emitted as E-matmuls interleaved with small "filler"
    units (projections, PV, transpose, C) kept under ~450ns each so the exp
    stream never starves; a budget scheduler pops fillers between E-matmuls.
  - PV in [q, 65] orientation (out free = 65 -> half the PE cost of the
    baseline's [65, q] form); a ones column in V gives softmax denominators.
  - o normalized per-q via DVE tensor_scalar with per-partition reciprocal,
    PE-transposed (bf16) to feature-major for the C matmul.
  - masked softmax: exp then tensor_tensor multiply with bf16 0/1 mask
    (DVE 2x_1p; 1/4 of tiles on GpSimd); max-subtraction and +eps dropped
    (|E|~<2; relative effect ~1e-11, same argument as the baseline).
  - OUT written bf16 (host accumulates partials in f32).
"""

import numpy as np
import ml_dtypes

import concourse.bacc as bacc
import concourse.bass as bass  # noqa: F401
import concourse.mybir as mybir
import concourse.tile as tile
from concourse.bass_utils import run_bass_kernel_spmd

B, N, D = 2, 2048, 1024
H = 16
HD = 64
HL = 4  # heads per core
DL = HL * HD  # 256 local features
P = 128
KO = D // P  # 8 contraction chunks for projections
NKC = N // P  # 16 k-token chunks
PAN = 1024  # q panel width in phase B
NPAN = N // PAN  # 2
QCP = PAN // P  # 8 q-chunks per panel
SCALE = 1.0 / 32.0  # 1/sqrt(DIM_V)

F32 = mybir.dt.float32
BF16 = mybir.dt.bfloat16
AF = mybir.ActivationFunctionType


def build_nc():
    nc = bacc.Bacc(None, target_bir_lowering=False)
    QT = nc.dram_tensor("qt", (D, N), BF16, kind="ExternalInput")
    KT = nc.dram_tensor("kt", (D, N), BF16, kind="ExternalInput")
    # mask, transposed + panel-major: [pan, p, kc, q'] = mask[b].T[kc*128+p, pan*1024+q']
    MT = nc.dram_tensor("mt", (NPAN, P, NKC, PAN), BF16, kind="ExternalInput")
    WQT = nc.dram_tensor("wqt", (D, DL), BF16, kind="ExternalInput")
    WKT = nc.dram_tensor("wkt", (D, DL), BF16, kind="ExternalInput")
    WVT = nc.dram_tensor("wvt", (D, DL), BF16, kind="ExternalInput")
    WOT = nc.dram_tensor("wot", (DL, D), BF16, kind="ExternalInput")
    BQ = nc.dram_tensor("bq", (DL,), F32, kind="ExternalInput")
    BK = nc.dram_tensor("bk", (DL,), F32, kind="ExternalInput")
    BV = nc.dram_tensor("bv", (DL,), F32, kind="ExternalInput")
    IDENT = nc.dram_tensor("ident", (P, P), BF16, kind="ExternalInput")
    OUT = nc.dram_tensor("out", (N, D), BF16, kind="ExternalOutput")

    qt_r = QT[:].rearrange("(ko p) q -> p ko q", p=P)
    kt_r = KT[:].rearrange("(ko p) q -> p ko q", p=P)

    uid = [0]

    def nm(pfx):
        uid[0] += 1
        return f"{pfx}{uid[0]}"

    with tile.TileContext(nc) as tc:
        with (
            tc.tile_pool(name="persist", bufs=1) as persist,
            tc.tile_pool(name="panpool", bufs=3) as panpool,
            tc.tile_pool(name="mpool", bufs=2) as mpool,
            tc.tile_pool(name="expool", bufs=4) as expool,
            tc.tile_pool(name="oqpool", bufs=2) as oqpool,
            tc.tile_pool(name="otpool", bufs=1) as otpool,
            tc.tile_pool(name="csb", bufs=2) as csb,
            tc.tile_pool(name="rcpool", bufs=2) as rcpool,
            tc.tile_pool(name="pepool", bufs=2, space="PSUM") as pepool,
            tc.tile_pool(name="cpool", bufs=2, space="PSUM") as cpool,
            tc.tile_pool(name="tpool", bufs=2, space="PSUM") as tpool,
        ):
            # ---- persistent SBUF ----
            qT = persist.tile([P, 2, N], BF16, tag="qT")
            kT = persist.tile([P, 2, N], BF16, tag="kT")
            v_sb = persist.tile([P, NKC, HL, HD + 1], BF16, tag="v")
            wq_sb = persist.tile([P, KO, DL], BF16, tag="wq")
            wk_sb = persist.tile([P, KO, DL], BF16, tag="wk")
            wv_sb = persist.tile([P, KO, DL], BF16, tag="wv")
            wo_sb = persist.tile([P, 2, D], BF16, tag="wo")
            bq_sb = persist.tile([P, 2], F32, tag="bq")
            bk_sb = persist.tile([P, 2], F32, tag="bk")
            bv_rep = persist.tile([P, HL, HD], F32, tag="bv")
            ident = persist.tile([P, P], BF16, tag="ident")

            m_half = {}

            def mask_dma(pan, half):
                """Mask half-panel [128, 8 kc, 1024]; two quad-DMAs."""
                m_half[(pan, half)] = mt = mpool.tile(
                    [P, NKC // 2, PAN], BF16, tag="m", name=f"m{pan}_{half}"
                )
                for qd in range(2):
                    nc.sync.dma_start(
                        out=mt[:, qd * 4 : (qd + 1) * 4, :],
                        in_=MT[pan, :, half * 8 + qd * 4 : half * 8 + (qd + 1) * 4, :],
                    )

            # ---- startup DMAs, ordered for earliest gapless exp stream ----
            def half_dma(t, src_r, col0, s):
                nc.sync.dma_start(
                    out=t[:, :, s * 512 : (s + 1) * 512],
                    in_=src_r[:, :, col0 + s * 512 : col0 + (s + 1) * 512],
                )

            nc.sync.dma_start(
                out=wk_sb[:], in_=WKT[:].rearrange("(ko p) m -> p ko m", p=P)
            )
            ktA = panpool.tile([P, KO, PAN], BF16, tag="pan", name="ktA")
            half_dma(ktA, kt_r, 0, 0)
            nc.sync.dma_start(
                out=wq_sb[:], in_=WQT[:].rearrange("(ko p) m -> p ko m", p=P)
            )
            nc.sync.dma_start(out=bk_sb[:], in_=BK[:].rearrange("(c p) -> p c", p=P))
            nc.sync.dma_start(out=bq_sb[:], in_=BQ[:].rearrange("(c p) -> p c", p=P))
            qt0 = panpool.tile([P, KO, PAN], BF16, tag="pan", name="qt0")
            half_dma(qt0, qt_r, 0, 0)
            half_dma(qt0, qt_r, 0, 1)
            half_dma(ktA, kt_r, 0, 1)
            ktB = panpool.tile([P, KO, PAN], BF16, tag="pan", name="ktB")
            half_dma(ktB, kt_r, PAN, 0)
            half_dma(ktB, kt_r, PAN, 1)
            nc.sync.dma_start(
                out=wv_sb[:], in_=WVT[:].rearrange("(ko p) m -> p ko m", p=P)
            )
            nc.sync.dma_start(
                out=bv_rep[:],
                in_=BV[:].rearrange("(h d) -> h d", h=HL)[None].to_broadcast(
                    (P, HL, HD)
                ),
            )
            mask_dma(0, 0)  # 2 quad DMAs
            mask_dma(0, 1)
            nc.vector.memset(v_sb[:, :, :, HD : HD + 1], 1.0)

            def late_dmas():
                nc.sync.dma_start(
                    out=wo_sb[:], in_=WOT[:].rearrange("(cc p) n -> p cc n", p=P)
                )
                nc.sync.dma_start(out=ident[:], in_=IDENT[:])

            # ---- helper emitters ----
            def proj_quarters(dst, fslice, w_sb, bias_ap, pan_cell, panslice):
                """One [128 x 512] projection as a list of 4 filler units
                (2 ko-steps each, ~426ns PE) sharing one cpool tile.
                pan_cell: 1-elem list deref'd lazily (reload tiles)."""
                cell = [None]

                def q_unit(qi):
                    if qi == 0:
                        cell[0] = cpool.tile([P, 512], F32, tag="c", name=nm("pq"))
                    ps = cell[0]
                    for ko in range(2 * qi, 2 * qi + 2):
                        nc.tensor.matmul(
                            ps[:],
                            lhsT=w_sb[:, ko, fslice],
                            rhs=pan_cell[0][:, ko, panslice],
                            start=(ko == 0),
                            stop=(ko == KO - 1),
                        )
                    if qi == 3:
                        nc.vector.tensor_scalar_add(dst, ps[:], bias_ap)

                return [(426, lambda qi=qi: q_unit(qi)) for qi in range(4)]

            def vproj_halves(pan_cell, t4, kc):
                """V for one 128-token chunk (all 4 heads) as 2 filler units."""
                cell = [None]

                def h_unit(hi):
                    if hi == 0:
                        cell[0] = cpool.tile([P, 512], F32, tag="c", name=nm("pq"))
                    ps = cell[0]
                    for ko in range(4 * hi, 4 * hi + 4):
                        nc.tensor.matmul(
                            ps[:, 0:DL],
                            lhsT=pan_cell[0][:, ko, t4 * P : (t4 + 1) * P],
                            rhs=wv_sb[:, ko, :],
                            start=(ko == 0),
                            stop=(ko == KO - 1),
                        )
                    if hi == 1:
                        nc.vector.tensor_add(
                            out=v_sb[:, kc, :, 0:HD],
                            in0=ps[:, 0:DL].rearrange("p (h d) -> p h d", h=HL),
                            in1=bv_rep[:],
                        )

                return [(428, lambda hi=hi: h_unit(hi)) for hi in range(2)]

            def pv_norm_unit(h, pan, qc, ex_lo, ex_hi, o_qd):
                """PV for one 128-q chunk + normalize into o_qd (bf16)."""
                ps = cpool.tile([P, 512], F32, tag="c")
                pso = ps[:, 0 : HD + 1]
                for kc in range(NKC):
                    exh = ex_lo if kc < 8 else ex_hi
                    nc.tensor.matmul(
                        pso,
                        lhsT=exh[:, kc % 8, qc * P : (qc + 1) * P],
                        rhs=v_sb[:, kc, h, :],
                        start=(kc == 0),
                        stop=(kc == NKC - 1),
                    )
                rcp = rcpool.tile([P, 1], F32, tag="rcp")
                nc.vector.reciprocal(out=rcp[:], in_=pso[:, HD : HD + 1])
                nc.vector.tensor_scalar(
                    out=o_qd[:, qc, h * HD : (h + 1) * HD],
                    in0=pso[:, 0:HD],
                    scalar1=rcp[:],
                    scalar2=None,
                    op0=mybir.AluOpType.mult,
                )

            def tc_units(pan, qc, o_qd, oT_pan, tail):
                """Transpose + C for one q-chunk, as 3 filler units.
                In the tail, one C eviction moves to the otherwise-idle Act."""
                dve_evict = lambda out, in_: nc.vector.tensor_copy(out=out, in_=in_)

                def t_unit():
                    for fc in range(2):
                        tp = tpool.tile([P, P], BF16, tag="t")
                        nc.tensor.transpose(
                            tp[:], o_qd[:, qc, fc * P : (fc + 1) * P], ident[:]
                        )
                        dve_evict(out=oT_pan[:, fc, qc * P : (qc + 1) * P], in_=tp[:])

                cell = [None]

                def c_unit(half):
                    if half == 0:
                        cell[0] = csb.tile([P, 2, 512], BF16, tag="co", name=nm("co"))
                    cp = cpool.tile([P, 512], F32, tag="c")
                    for fc in range(2):
                        nc.tensor.matmul(
                            cp[:],
                            lhsT=oT_pan[:, fc, qc * P : (qc + 1) * P],
                            rhs=wo_sb[:, fc, half * 512 : (half + 1) * 512],
                            start=(fc == 0),
                            stop=(fc == 1),
                        )
                    if tail and half == 1:
                        nc.scalar.copy(out=cell[0][:, half, :], in_=cp[:])
                    else:
                        dve_evict(out=cell[0][:, half, :], in_=cp[:])
                    if half == 1:
                        q0 = pan * PAN + qc * P
                        nc.sync.dma_start(out=OUT[q0 : q0 + P, :], in_=cell[0][:])

                return [(120, t_unit), (426, lambda: c_unit(0)), (426, lambda: c_unit(1))]

            # ---- startup projections: K(dc0) tokens 0..511, Q(p0,dc0) ----
            for est, f in proj_quarters(
                kT[:, 0, 0:512], slice(0, P), wk_sb,
                bk_sb[:, 0:1], [ktA], slice(0, 512),
            ):
                f()
            for u in range(2):
                for est, f in proj_quarters(
                    qT[:, 0, u * 512 : (u + 1) * 512], slice(0, P), wq_sb,
                    bq_sb[:, 0:1], [qt0], slice(u * 512, (u + 1) * 512),
                ):
                    f()

            # K(dc0) for tokens 1024..2047 trails into the B loop as fillers
            fillers = []
            est_state = {"pe": 18000.0, "act": 0.0, "hist": []}

            def addf(units, key=None):
                for est, f in units:
                    fillers.append((est, key, f))

            addf(
                proj_quarters(
                    kT[:, 0, 512:1024], slice(0, P),
                    wk_sb, bk_sb[:, 0:1], [ktA], slice(512, 1024),
                ),
                key="kdc0a",
            )
            addf(
                proj_quarters(
                    kT[:, 0, PAN : PAN + 512], slice(0, P),
                    wk_sb, bk_sb[:, 0:1], [ktB], slice(0, 512),
                ),
                key="kdc0b1",
            )
            # Q(p0, dc1)
            for u in range(2):
                addf(
                    proj_quarters(
                        qT[:, 1, u * 512 : (u + 1) * 512], slice(P, 2 * P), wq_sb,
                        bq_sb[:, 1:2], [qt0], slice(u * 512, (u + 1) * 512),
                    ),
                    key="qp0",
                )
            # V + K(dc1) straight from the resident ktA/ktB panels
            for t4 in range(8):
                addf(vproj_halves([ktA], t4, t4), key="vk")
                if t4 == 1:
                    # second half of K(dc0) (needs the later ktB half-DMA)
                    addf(
                        proj_quarters(
                            kT[:, 0, PAN + 512 : N], slice(0, P),
                            wk_sb, bk_sb[:, 0:1], [ktB], slice(512, 1024),
                        ),
                        key="kdc0b2",
                    )
                if t4 == 3:
                    addf(
                        proj_quarters(
                            kT[:, 1, 0:512], slice(P, 2 * P), wk_sb,
                            bk_sb[:, 1:2], [ktA], slice(0, 512),
                        ),
                        key="vk",
                    )
            addf(
                proj_quarters(
                    kT[:, 1, 512:1024], slice(P, 2 * P), wk_sb, bk_sb[:, 1:2],
                    [ktA], slice(512, 1024),
                ),
                key="vk",
            )
            for t4 in range(8):
                addf(vproj_halves([ktB], t4, 8 + t4), key="vkb")
                if t4 == 3:
                    addf(
                        proj_quarters(
                            kT[:, 1, PAN : PAN + 512], slice(P, 2 * P), wk_sb,
                            bk_sb[:, 1:2], [ktB], slice(0, 512),
                        ),
                        key="vkb",
                    )
            addf(
                proj_quarters(
                    kT[:, 1, PAN + 512 : N], slice(P, 2 * P), wk_sb, bk_sb[:, 1:2],
                    [ktB], slice(512, 1024),
                ),
                key="vkb",
            )
            # qt panel 1 (4th 'pan' ring alloc frees ktA) + Q(p1) both dc
            qt1 = [None]

            def alloc_dma_qt1():
                qt1[0] = t = panpool.tile([P, KO, PAN], BF16, tag="pan", name="qt1")
                for s in range(2):
                    nc.sync.dma_start(
                        out=t[:, :, s * 512 : (s + 1) * 512],
                        in_=qt_r[:, :, PAN + s * 512 : PAN + (s + 1) * 512],
                    )

            addf([(0, alloc_dma_qt1)], key="qp1")
            for dc in range(2):
                for u in range(2):
                    addf(
                        proj_quarters(
                            qT[:, dc, PAN + u * 512 : PAN + (u + 1) * 512],
                            slice(dc * P, (dc + 1) * P), wq_sb,
                            bq_sb[:, dc : dc + 1], qt1, slice(u * 512, (u + 1) * 512),
                        ),
                        key="qp1",
                    )
            addf([(0, late_dmas)])

            def emit_fillers():
                """Pop fillers while the estimated PE clock trails the
                estimated Act clock (keeps exp gapless without bursting)."""
                while fillers:
                    est, key, f = fillers[0]
                    if est_state["pe"] + est > est_state["act"] - 900:
                        break
                    fillers.pop(0)
                    f()
                    est_state["pe"] += est

            def drain_through(keys):
                """Force-emit queue entries up to the last entry whose key is
                in `keys` (emission-order deadlines at head boundaries)."""
                if not any(k in keys for _, k, _ in fillers):
                    return
                last = max(i for i, (_, k, _) in enumerate(fillers) if k in keys)
                for est, _, f in fillers[: last + 1]:
                    f()
                    est_state["pe"] += est
                del fillers[: last + 1]

            # ---- fused B/C pipeline ----
            # ex half-tile slot sources per global head: 'x'=expool(4-ring),
            # 'p'=panpool 'pan' slot (same 16KB shape; kt/qt panels dead by
            # then). This 7-slot rotation removes the PV(h-2)-before-exp(h)
            # deadline of the old 2-deep full-tile ring.
            EX_SRC = [
                ("x", "x"), ("x", "x"), ("p", "p"), ("x", "x"),
                ("x", "x"), ("p", "p"), ("p", "x"), ("x", "x"),
            ]
            pending = []
            for pan in range(NPAN):
                o_qd = oqpool.tile([P, QCP, DL], BF16, tag="oq", name=f"oq{pan}")
                oT_pan = otpool.tile([P, 2, PAN], BF16, tag="ot", name=f"ot{pan}")
                for h in range(HL):
                    g = pan * HL + h
                    dc, po = h // 2, (h % 2) * HD
                    # emission-order deadlines (before this head's first E):
                    # kT(dc1) writers before h2; qT(p1) before panel 1.
                    need = set()
                    if pan == 0 and h == 2:
                        need.add("vk")
                    if pan == 0 and h == 3:
                        need.add("vkb")
                    if pan == 1 and h == 0:
                        need.add("qp1")
                    drain_through(need)
                    deferred_vkb = pan == 0 and h == 2
                    src_lo, src_hi = EX_SRC[g]
                    def _ex_alloc(srcc, half):
                        if srcc == "x":
                            return expool.tile(
                                [P, NKC // 2, PAN], BF16, tag="exh",
                                name=f"ex{pan}_{h}_{half}",
                            )
                        return panpool.tile(
                            [P, KO, PAN], BF16, tag="pan", name=f"exp{pan}_{h}_{half}"
                        )
                    ex_lo = _ex_alloc(src_lo, 0)
                    ex_hi = _ex_alloc(src_hi, 1)
                    for kc in range(NKC):
                        ps = pepool.tile([P, PAN], F32, tag="e")
                        for eh in range(2):
                            nc.tensor.matmul(
                                ps[:, eh * 512 : (eh + 1) * 512],
                                lhsT=kT[po : po + HD, dc, kc * P : (kc + 1) * P],
                                rhs=qT[
                                    po : po + HD,
                                    dc,
                                    pan * PAN + eh * 512 : pan * PAN + (eh + 1) * 512,
                                ],
                                start=True,
                                stop=True,
                            )
                        hist = est_state["hist"]
                        if len(hist) >= 2:
                            est_state["pe"] = max(est_state["pe"], hist[-2] + 250)
                        est_state["pe"] += 427
                        exh = ex_lo if kc < 8 else ex_hi
                        nc.scalar.activation(
                            out=exh[:, kc % 8, :], in_=ps[:], func=AF.Exp, scale=SCALE
                        )
                        prev = hist[-1] if hist else est_state["pe"] + 870
                        hist.append(max(prev, est_state["pe"] + 250) + 1038)
                        est_state["act"] = hist[-1]
                        if kc == 3 and pending:
                            vk_idx = [
                                i
                                for i, (_, k, _) in enumerate(fillers)
                                if k in ("vk", "vkb")
                            ]
                            at = (vk_idx[-1] + 1) if vk_idx else 0
                            fillers[at:at] = pending
                            pending.clear()
                        if pan == 0 and h == 0 and kc == 2:
                            drain_through({"kdc0a"})
                        if pan == 0 and h == 0 and kc == 6:
                            drain_through({"kdc0b1", "kdc0b2"})
                        if pan == 0 and h == 1 and kc == 0:
                            drain_through({"kdc0b1", "kdc0b2"})
                        if kc == 2 and g >= 2:
                            drain_through({("pv", g - 2)})

                        if deferred_vkb and kc == 5:
                            drain_through({"vkb"})
                        emit_fillers()
                        last_head = pan == NPAN - 1 and h == HL - 1
                        exh = ex_lo if kc < 8 else ex_hi
                        if last_head and kc >= 14:
                            mt = m_half[(pan, 1)]
                            k0 = kc - 8
                            nc.vector.tensor_mul(
                                out=exh[:, k0 : k0 + 1, :],
                                in0=exh[:, k0 : k0 + 1, :],
                                in1=mt[:, k0 : k0 + 1, :],
                            )
                        elif kc % 2 == 1:
                            mt = m_half[(pan, kc // 8)]
                            k0 = kc - 1 - (kc // 8) * 8
                            nc.vector.tensor_mul(
                                out=exh[:, k0 : k0 + 2, :],
                                in0=exh[:, k0 : k0 + 2, :],
                                in1=mt[:, k0 : k0 + 2, :],
                            )
                        if pan == 0 and h == 3 and kc == 2:
                            mask_dma(1, 0)
                        if pan == 0 and h == 3 and kc == 10:
                            mask_dma(1, 1)
                    # queue this head's PV into `pending`: released at kc==3
                    # of the next head, after that head's last masks have had
                    # time to finish (PV's kc14/15 matmuls depend on them)
                    for qc in range(QCP):
                        pending.append(
                            (
                                432,
                                ("pv", g),
                                lambda h=h, pan=pan, qc=qc, el=ex_lo, eh2=ex_hi, o_qd=o_qd: pv_norm_unit(
                                    h, pan, qc, el, eh2, o_qd
                                ),
                            )
                        )
                    if h == HL - 1:
                        for qc in range(QCP):
                            for est, f in tc_units(
                                pan, qc, o_qd, oT_pan, tail=(pan == NPAN - 1)
                            ):
                                pending.append((est, "tc", f))

            # drain remaining fillers; software-pipeline the last panel's
            # PV -> T -> C chains (PV j+1 before T j, C j-1 after T j)
            while fillers:
                fillers.pop(0)[2]()
            pvs = [f for _, k, f in pending if k != "tc"]
            tcs = [f for _, k, f in pending if k == "tc"]
            tcg = [tcs[i * 3 : (i + 1) * 3] for i in range(len(tcs) // 3)]
            pending.clear()
            stages = []
            for j in range(len(pvs)):
                stages.append(pvs[j])
                if j >= 1 and j - 1 < len(tcg):
                    stages.extend(tcg[j - 1][:1])  # transpose of j-1
                if j >= 2 and j - 2 < len(tcg):
                    stages.extend(tcg[j - 2][1:])  # C halves of j-2
            for j in range(max(0, len(pvs) - 1), len(tcg)):
                stages.extend(tcg[j][:1])
            for j in range(max(0, len(pvs) - 2), len(tcg)):
                stages.extend(tcg[j][1:])
            for f in stages:
                f()

    nc.finalize()
    return nc


_NC = None


def _get_nc():
    global _NC
    if _NC is None:
        _NC = build_nc()
    return _NC


def make_in_maps(Q, K, mask, Wq, bq, Wk, bk, Wv, bv, Wo, bo):
    Q = np.asarray(Q, np.float32)
    K = np.asarray(K, np.float32)
    mask = np.asarray(mask)
    Wq = np.asarray(Wq, np.float32)
    Wk = np.asarray(Wk, np.float32)
    Wv = np.asarray(Wv, np.float32)
    Wo = np.asarray(Wo, np.float32)
    qt = [np.ascontiguousarray(Q[b].T).astype(ml_dtypes.bfloat16) for b in range(B)]
    kt = [np.ascontiguousarray(K[b].T).astype(ml_dtypes.bfloat16) for b in range(B)]
    # [pan, p, kc, q'] = mask[b].T[kc*128+p, pan*1024+q']
    mt = [
        np.ascontiguousarray(
            mask[b].T.reshape(NKC, P, NPAN, PAN).transpose(2, 1, 0, 3)
        ).astype(ml_dtypes.bfloat16)
        for b in range(B)
    ]
    ident = np.eye(P, dtype=np.float32).astype(ml_dtypes.bfloat16)
    in_maps = []
    for c in range(8):
        b, hg = divmod(c, 4)
        cols = slice(hg * DL, (hg + 1) * DL)
        in_maps.append(
            {
                "qt": qt[b],
                "kt": kt[b],
                "mt": mt[b],
                "wqt": np.ascontiguousarray(Wq[cols, :].T).astype(ml_dtypes.bfloat16),
                "wkt": np.ascontiguousarray(Wk[cols, :].T).astype(ml_dtypes.bfloat16),
                "wvt": np.ascontiguousarray(Wv[cols, :].T).astype(ml_dtypes.bfloat16),
                "wot": np.ascontiguousarray(Wo[:, cols].T).astype(ml_dtypes.bfloat16),
                "bq": np.ascontiguousarray(np.asarray(bq, np.float32)[cols]),
                "bk": np.ascontiguousarray(np.asarray(bk, np.float32)[cols]),
                "bv": np.ascontiguousarray(np.asarray(bv, np.float32)[cols]),
                "ident": ident,
            }
        )
    return in_maps


def assemble(results, bo):
    O = np.zeros((B, N, D), np.float32)
    for c in range(8):
        b = c // 4
        O[b] += np.asarray(results[c]["out"], np.float32)
    O += np.asarray(bo, np.float32)[None, None, :]
    return O


def kernel(Q, K, mask, Wq, bq, Wk, bk, Wv, bv, Wo, bo):
    nc = _get_nc()
    in_maps = make_in_maps(Q, K, mask, Wq, bq, Wk, bk, Wv, bv, Wo, bo)
    res = run_bass_kernel_spmd(nc, in_maps, core_ids=list(range(8)))
    return assemble(res.results, bo)
